# revision 21
# baseline (speedup 1.0000x reference)
"""Trainium2 Bass kernel for nn_MultiHeadCrossAttention (BS=4, S=512, DM=512, H=8).

Sharding: one attention head per NeuronCore (8 heads / 8 cores). Each core
receives the full (transposed) q/k/v plus its head's weight slices, computes
its head end-to-end including the rank-64 slice of the output projection, and
the host sums the 8 partial outputs.

v2 layout/engine plan:
  - weights packed into one bf16 blob + one f32 blob, DMA'd first
  - k/q projections emit [128,*] dup outputs (lhsT = [W|W]) so P2 can run
    kb-pairs concurrently in row groups {0-63} / {64-127}
  - fenmu = sum_kb exp(.) via bf16 identity-matmul PSUM accumulation
  - softmax/LN stats via grouped bn_stats (even/odd combine) + 1-NR rsqrt
  - w0 applied pre-transpose as a per-partition scalar (no rank-1 matmuls)
  - P6 single K=128 matmul per chunk: lhsT = [Wo;4Wo], rhs = [heads;qh]
  - output DRAM layout [B,128,4,S] (4KB/partition descriptors)
"""

import numpy as np

BS, S, DM, H, DK = 4, 512, 512, 8, 64
NCORES = 8
WCOLS = 512 + 512 + 256 + 512 + 64 + 128  # Wk,Wq,Wv,WoB,bvb,id = 1984
FCOLS = 8 + 128  # biases + idf


def build_program(nc, tile, mybir):
    f32 = mybir.dt.float32
    bf16 = mybir.dt.bfloat16
    i32 = mybir.dt.int32
    AF = mybir.ActivationFunctionType
    OP = mybir.AluOpType

    wb_d = nc.dram_tensor("wblob", [128, WCOLS], bf16, kind="ExternalInput")
    fb_d = nc.dram_tensor("fblob", [128, FCOLS], f32, kind="ExternalInput")
    kT_d = nc.dram_tensor("kT", [BS, 128, 4, S], bf16, kind="ExternalInput")
    qT_d = nc.dram_tensor("qT", [BS, 128, 4, S], bf16, kind="ExternalInput")
    vT_d = nc.dram_tensor("vT", [BS, 128, 4, S], bf16, kind="ExternalInput")
    outT_d = nc.dram_tensor("outT", [BS, 128, 4, S], bf16, kind="ExternalOutput")

    with tile.TileContext(nc) as tc:
        with (
            tc.tile_pool(name="persist", bufs=1) as pp,
            tc.tile_pool(name="inp", bufs=3) as inp,
            tc.tile_pool(name="ex", bufs=2) as exp_,
            tc.tile_pool(name="wk", bufs=2) as wkp,
            tc.tile_pool(name="st", bufs=1) as stp,
            tc.tile_pool(name="ob", bufs=2) as obp,
            tc.tile_pool(name="psum", bufs=1, space="PSUM") as psp,
        ):
            # ---- persistent SBUF ----
            wb = pp.tile([128, WCOLS], bf16, tag="wb")
            fb = pp.tile([128, FCOLS], f32, tag="fb")
            Wk_s = wb[:, 0:512].rearrange("p (m c) -> p m c", m=4)
            Wq_s = wb[:, 512:1024].rearrange("p (m c) -> p m c", m=4)
            Wv_s = wb[:, 1024:1280].rearrange("p (m c) -> p m c", m=4)
            WoB_s = wb[:, 1280:1792].rearrange("p (m c) -> p m c", m=4)
            bvb_s = wb[:, 1792:1856]
            id_s = wb[:, 1856:1984]
            bk2 = fb[:, 0:1]
            bq2 = fb[:, 1:2]
            bo4 = fb[:, 2:6]
            alx = fb[:, 6:7]
            b4x = fb[:, 7:8]
            idf_s = fb[:, 8:136]

            khT2 = pp.tile([128, BS, S], bf16, tag="khT2")
            qhT2 = pp.tile([128, BS, S], bf16, tag="qhT2")
            vh_all = pp.tile([128, 4, BS, DK], bf16, tag="vh")  # [j,jc,c,d]
            heads = pp.tile([128, BS, S], bf16, tag="heads")  # 0:64 ln, 64: qh
            Z_all = pp.tile([128, BS, 16], f32, tag="Z")   # (b, ic*4+c)
            Q_all = pp.tile([128, BS, 16], f32, tag="Q")
            w1_all = pp.tile([128, BS, 16], f32, tag="w1")
            w0_all = pp.tile([128, 16], bf16, tag="w0")
            wup = pp.tile([1, 8], f32, tag="wup")

            def psum_pe():
                return psp.tile([128, 2, S], f32, tag="pe", bufs=2, name="ppe")

            def psum_fs(shape):
                return psp.tile(shape, f32, tag="fs", bufs=2, name="pfs")

            def psum_po(shape):
                return psp.tile(shape, f32, tag="po", bufs=2, name="ppo")

            # ---- prelude: ACT table preload + DMAs in use-order ----
            nc.vector.memset(wup[:], 1.0)
            nc.scalar.activation(wup[:], wup[:], AF.Exp)
            nc.sync.dma_start(wb[:], wb_d[:])
            ktiles = [inp.tile([128, 4, S], bf16, tag="kt", bufs=4,
                               name=f"kt{b}") for b in range(BS)]
            qtiles = [inp.tile([128, 4, S], bf16, tag="qt", bufs=4,
                               name=f"qt{b}") for b in range(BS)]
            vtiles = [inp.tile([128, 4, S], bf16, tag="vt", bufs=4,
                               name=f"vt{b}") for b in range(BS)]
            # trigger order = sync-engine issue order: wblob first, then k
            # (P2 needs all of kh), q0, then v and the remaining q's.
            nc.sync.dma_start(ktiles[0][:], kT_d[0])
            nc.sync.dma_start(ktiles[1][:], kT_d[1])
            nc.sync.dma_start(qtiles[0][:], qT_d[0])
            nc.sync.dma_start(ktiles[2][:], kT_d[2])
            nc.sync.dma_start(ktiles[3][:], kT_d[3])
            nc.sync.dma_start(qtiles[1][:], qT_d[1])
            nc.sync.dma_start(fb[:], fb_d[:])
            nc.sync.dma_start(qtiles[2][:], qT_d[2])
            nc.sync.dma_start(qtiles[3][:], qT_d[3])
            nc.sync.dma_start(vtiles[0][:], vT_d[0])
            nc.sync.dma_start(vtiles[1][:], vT_d[1])
            nc.sync.dma_start(vtiles[2][:], vT_d[2])
            nc.sync.dma_start(vtiles[3][:], vT_d[3])

            # ---- phase emitters ----
            def emit_p1(W_s, bias_c, src, dst, b):
                ps = psum_po([128, S])
                for mc in range(4):
                    nc.tensor.matmul(ps[:], W_s[:, mc, :], src[:, mc, :],
                                     start=(mc == 0), stop=(mc == 3))
                nc.vector.tensor_scalar(dst[:, b, :], ps[:], bias_c, None,
                                        op0=OP.add)

            def emit_hq(b):
                nc.vector.tensor_copy(heads[64:128, b, :], qhT2[64:128, b, :])

            def emit_vh(c):
                pv = psum_po([128, 4, DK])
                vt = vtiles[c]
                for jc in range(4):
                    for mc in range(4):
                        nc.tensor.matmul(
                            pv[:, jc, :], vt[:, mc, jc * 128:(jc + 1) * 128],
                            Wv_s[:, mc, :], start=(mc == 0), stop=(mc == 3))
                nc.vector.tensor_tensor(
                    vh_all[:, :, c, :], pv[:],
                    bvb_s[:].unsqueeze(1).broadcast_to((128, 4, DK)),
                    op=OP.add)

            rtiles = {}

            def emit_p2(b, jc):
                jcs = slice(jc * 128, (jc + 1) * 128)
                exq = exp_.tile([128, 4, S], bf16, tag="ex", name="exq")
                for half in range(2):
                    ph = psum_pe()
                    kb0, kb1 = 2 * half, 2 * half + 1
                    nc.tensor.matmul(ph[:, 0, :], khT2[0:64, kb0, jcs],
                                     qhT2[0:64, b, :], start=True, stop=True)
                    nc.tensor.matmul(ph[:, 1, :], khT2[64:128, kb1, jcs],
                                     qhT2[64:128, b, :], start=True, stop=True)
                    nc.scalar.activation(
                        exq[:, 2 * half:2 * half + 2, :], ph[:], AF.Exp)
                fp = psum_fs([128, S])
                for kb in range(4):
                    nc.tensor.matmul(fp[:], id_s[:], exq[:, kb, :],
                                     start=(kb == 0), stop=(kb == 3))
                wrec = wkp.tile([128, S], f32, tag="wrec", name="wrec")
                nc.vector.reciprocal_approx_fast(wrec[:], fp[:])
                if jc == 0:
                    rtiles[b] = wkp.tile([128, 4, S], bf16, tag="rt",
                                         bufs=4, name=f"rt{b}")
                nc.gpsimd.tensor_tensor(rtiles[b][:, jc, :], exq[:, b, :],
                                        wrec[:], op=OP.mult)

            etiles = {}

            def emit_p3(b, p):
                rt = rtiles[b]
                if p == 0:
                    etiles[b] = wkp.tile([128, 4, BS, DK], bf16, tag="ea",
                                         bufs=3, name=f"ea{b}")
                ea = etiles[b]
                scp = psum_fs([128, 2, BS * DK])
                for u in range(2):
                    ic = 2 * p + u
                    ics = slice(ic * 128, (ic + 1) * 128)
                    for jc in range(4):
                        nc.tensor.matmul(
                            scp[:, u, :], rt[:, jc, ics],
                            vh_all[:, jc].rearrange("p c d -> p (c d)"),
                            start=(jc == 0), stop=(jc == 3))
                nc.scalar.activation(
                    ea[:, 2 * p:2 * p + 2].rearrange("p a c d -> p a (c d)"),
                    scp[:], AF.Exp)

            def emit_zq(b, eng):
                # e2 = e*e (gpsimd), then grouped reduces -> Z, Q (DVE-only)
                ea = etiles[b]
                e2 = wkp.tile([128, 16, DK], bf16, tag="e2", name="e2")
                eav = ea[:].rearrange("p a c d -> p (a c) d")
                eng.tensor_tensor(e2[:], eav, eav, op=OP.mult)
                nc.vector.tensor_reduce(Z_all[:, b, :], eav,
                                        axis=mybir.AxisListType.X, op=OP.add)
                nc.vector.tensor_reduce(Q_all[:, b, :], e2[:],
                                        axis=mybir.AxisListType.X, op=OP.add)

            SQ63 = float(np.sqrt(63.0))

            def emit_stats(b0, nb):
                # batches [b0, b0+nb): ln scale-invariance kills 1/Z:
                # w1 = sqrt(63)*rsqrt(Q - Z^2/64), w0 = -Z*w1/64
                bsl = slice(b0, b0 + nb)
                cnt = [128, 16 * nb]
                Zv = Z_all[:, bsl, :].rearrange("p b g -> p (b g)")
                Qv = Q_all[:, bsl, :].rearrange("p b g -> p (b g)")
                t_ = stp.tile(cnt, f32, tag=f"t{b0}", name="t_")
                nc.vector.tensor_tensor(t_[:], Zv, Zv, op=OP.mult)
                s_ = stp.tile(cnt, f32, tag=f"s{b0}", name="s_")
                nc.vector.scalar_tensor_tensor(s_[:], t_[:], -1.0 / DK, Qv,
                                               op0=OP.mult, op1=OP.add)
                # rsqrt seed + 1 NR iter (w1 fused into the final mult)
                r_ = stp.tile(cnt, f32, tag=f"r{b0}", name="r_")
                nc.vector.tensor_scalar(r_[:].bitcast(i32), s_[:].bitcast(i32),
                                        1, None, op0=OP.logical_shift_right)
                nc.vector.tensor_scalar(r_[:].bitcast(i32), r_[:].bitcast(i32),
                                        -1, 0x5F3759DF, op0=OP.mult, op1=OP.add)
                nt = stp.tile(cnt, f32, tag=f"n{b0}", name="nt")
                nc.vector.tensor_tensor(nt[:], s_[:], r_[:], op=OP.mult)
                nc.vector.tensor_tensor(nt[:], nt[:], r_[:], op=OP.mult)
                nc.vector.tensor_scalar(nt[:], nt[:], -0.5, 1.5,
                                        op0=OP.mult, op1=OP.add)
                w1v = w1_all[:, bsl, :].rearrange("p b g -> p (b g)")
                nc.vector.scalar_tensor_tensor(w1v, r_[:], SQ63, nt[:],
                                               op0=OP.mult, op1=OP.mult)
                zg = stp.tile(cnt, f32, tag=f"zg{b0}", name="zg")
                nc.vector.scalar_tensor_tensor(zg[:], Zv, -1.0 / DK, w1v,
                                               op0=OP.mult, op1=OP.mult)
                w0r = stp.tile([128, 4 * nb], f32, tag=f"w{b0}", name="w0r")
                nc.vector.tensor_reduce(
                    w0r[:], zg[:].rearrange("p (g c) -> p g c", c=4),
                    axis=mybir.AxisListType.X, op=OP.add)
                nc.vector.tensor_copy(w0_all[:, 4 * b0:4 * (b0 + nb)], w0r[:])

            bsctiles = {}

            def emit_bsc(b, eng):
                ea = etiles[b]
                bsc = obp.tile([128, 16, DK], bf16, tag="bsc", bufs=4,
                               name=f"bsc{b}")
                bsctiles[b] = bsc
                w1b = (w1_all[:, b, :].unsqueeze(-1)
                       .broadcast_to((128, 16, DK)))
                eng.tensor_tensor(
                    bsc[:], ea[:].rearrange("p a c d -> p (a c) d"), w1b,
                    op=OP.mult)

            def emit_p5(b):
                bsc = bsctiles[b]
                bp = psum_fs([128, 4, DK])
                for ic in range(4):
                    for c in range(4):
                        nc.tensor.matmul(bp[:, ic, :], id_s[:],
                                         bsc[:, ic * 4 + c, :],
                                         start=(c == 0), stop=False)
                    # += w0 broadcast along d via a 0-stride rhs
                    nc.tensor.matmul(
                        bp[:, ic, :], id_s[:],
                        w0_all[:, b * 4 + ic:b * 4 + ic + 1]
                        .broadcast_to((128, DK)),
                        start=False, stop=True)
                balls = obp.tile([128, 4, DK], f32, tag="balls", name="balls")
                nc.vector.tensor_copy(balls[:], bp[:])
                pt = psum_po([64, S])
                for ic in range(4):
                    nc.tensor.matmul(pt[0:64, ic * 128:(ic + 1) * 128],
                                     balls[:, ic, :], idf_s,
                                     is_transpose=True, start=True, stop=True)
                nc.vector.tensor_scalar(heads[0:64, b, :], pt[0:64, :],
                                        alx[0:64, :], b4x[0:64, :],
                                        op0=OP.mult, op1=OP.add)

            def emit_p6(b):
                osb = obp.tile([128, 4, S], bf16, tag="osb", name="osb")
                for nch in range(4):
                    pp6 = psum_po([128, S])
                    nc.tensor.matmul(pp6[:], WoB_s[:, nch, :], heads[:, b, :],
                                     start=True, stop=True)
                    if nch % 2 == 0:
                        nc.scalar.activation(osb[:, nch, :], pp6[:],
                                             AF.Identity,
                                             bias=bo4[:, nch:nch + 1])
                    else:
                        nc.vector.tensor_scalar(osb[:, nch, :], pp6[:],
                                                bo4[:, nch:nch + 1], None,
                                                op0=OP.add)
                nc.sync.dma_start(outT_d[b], osb[:])

            # ---- emission schedule (engine queues are FIFO: never emit
            # work whose inputs arrive later than the next emission's) ----
            emit_p1(Wk_s, bk2, ktiles[0], khT2, 0)
            emit_p1(Wk_s, bk2, ktiles[1], khT2, 1)
            emit_p1(Wq_s, bq2, qtiles[0], qhT2, 0)
            emit_p1(Wk_s, bk2, ktiles[2], khT2, 2)
            emit_p1(Wk_s, bk2, ktiles[3], khT2, 3)
            emit_hq(0)
            emit_p2(0, 0)
            emit_p1(Wq_s, bq2, qtiles[1], qhT2, 1)
            emit_hq(1)
            emit_p2(0, 1)
            emit_p2(0, 2)
            emit_p1(Wq_s, bq2, qtiles[2], qhT2, 2)
            emit_hq(2)
            emit_p2(0, 3)
            emit_p2(1, 0)
            emit_p1(Wq_s, bq2, qtiles[3], qhT2, 3)
            emit_hq(3)
            emit_p2(1, 1)
            emit_p2(1, 2)
            emit_p2(1, 3)
            emit_vh(0)
            emit_p2(2, 0)
            emit_vh(1)
            emit_p2(2, 1)
            emit_vh(2)
            emit_p2(2, 2)
            emit_vh(3)
            emit_p2(2, 3)
            emit_p2(3, 0)
            emit_p3(0, 0)
            emit_p2(3, 1)
            emit_p3(0, 1)
            emit_p2(3, 2)
            emit_zq(0, nc.vector)
            emit_p3(1, 0)
            emit_p2(3, 3)
            emit_p3(1, 1)
            emit_zq(1, nc.vector)
            emit_stats(0, 2)
            emit_bsc(0, nc.gpsimd)
            emit_bsc(1, nc.gpsimd)
            emit_p3(2, 0)
            emit_p3(2, 1)
            emit_zq(2, nc.vector)
            emit_stats(2, 1)
            emit_bsc(2, nc.gpsimd)
            emit_p3(3, 0)
            emit_p3(3, 1)
            emit_zq(3, nc.vector)
            emit_stats(3, 1)
            emit_bsc(3, nc.vector)
            emit_p5(0)
            emit_p6(0)
            emit_p5(1)
            emit_p6(1)
            emit_p5(2)
            emit_p6(2)
            emit_p5(3)
            emit_p6(3)

    return nc


def _build():
    import concourse.bass as bass  # noqa
    import concourse.tile as tile
    from concourse import bacc, mybir

    nc = bacc.Bacc("TRN2", target_bir_lowering=False, debug=False,
                   num_devices=NCORES)
    build_program(nc, tile, mybir)
    nc.compile()
    return nc


_cached_nc = None


def make_in_maps(q, k, v, Wq, bq, Wk, bk, Wv, bv, Wo, bo, alpha, beta):
    import ml_dtypes
    bft = ml_dtypes.bfloat16

    def prelay(x):
        xT = np.swapaxes(np.asarray(x, np.float32), 1, 2)  # [B, DM, S]
        return np.ascontiguousarray(
            xT.reshape(BS, 4, 128, S).transpose(0, 2, 1, 3)).astype(bft)

    def wlay(W):  # [DM, DK] -> [128, 4, DK]
        return np.ascontiguousarray(
            np.asarray(W, np.float32).reshape(4, 128, DK).transpose(1, 0, 2))

    qT, kT, vT = prelay(q), prelay(k), prelay(v)
    Wq, Wk, Wv, Wo = (np.asarray(x, np.float32) for x in (Wq, Wk, Wv, Wo))
    bq, bk, bv, bo = (np.asarray(x, np.float32) for x in (bq, bk, bv, bo))
    alpha, beta = np.asarray(alpha, np.float32), np.asarray(beta, np.float32)
    scale = np.float32(1.0 / np.sqrt(np.float32(DK)))
    idbf = np.eye(128, dtype=np.float32)
    in_maps = []
    for h in range(NCORES):
        sl = slice(h * DK, (h + 1) * DK)
        WkD = wlay(Wk[:, sl])
        WkD = np.concatenate([WkD, WkD], axis=2).reshape(128, 512)
        WqD = wlay(Wq[:, sl])
        WqD = np.concatenate([WqD, WqD], axis=2).reshape(128, 512)
        WvS = wlay(Wv[:, sl] * scale).reshape(128, 256)
        WoStack = np.concatenate([Wo[sl, :], 4.0 * Wo[sl, :]], axis=0)
        WoB = WoStack.reshape(128, 4, 128).reshape(128, 512)
        bvb = np.tile((bv[sl] * scale)[None, :], (128, 1))
        wblob = np.ascontiguousarray(np.concatenate(
            [WkD, WqD, WvS, WoB, bvb, idbf], axis=1)).astype(bft)
        bo_h = bo if h == 0 else np.zeros_like(bo)
        alx = np.zeros(128, np.float32)
        alx[0:DK] = alpha
        b4x = np.zeros(128, np.float32)
        b4x[0:DK] = 4.0 * beta
        fblob = np.ascontiguousarray(np.concatenate(
            [np.tile(bk[sl], 2)[:, None], np.tile(bq[sl], 2)[:, None],
             bo_h.reshape(4, 128).T, alx[:, None], b4x[:, None], idbf],
            axis=1)).astype(np.float32)
        in_maps.append({
            "wblob": wblob, "fblob": fblob,
            "kT": kT, "qT": qT, "vT": vT,
        })
    return in_maps


def assemble(results):
    out = np.zeros((BS, S, DM), np.float32)
    for r in results:
        out += np.asarray(r["outT"], np.float32).transpose(0, 3, 2, 1).reshape(
            BS, S, DM)
    return out


def kernel(**inputs) -> np.ndarray:
    global _cached_nc
    from concourse.bass_utils import run_bass_kernel_spmd

    if _cached_nc is None:
        _cached_nc = _build()
    in_maps = make_in_maps(**inputs)
    res = run_bass_kernel_spmd(_cached_nc, in_maps, list(range(NCORES)))
    return assemble(res.results)


# revision 23
# speedup vs baseline: 1.0621x; 1.0621x over previous
"""Trainium2 Bass kernel for nn_MultiHeadCrossAttention (BS=4, S=512, DM=512, H=8).

Sharding: one attention head per NeuronCore (8 heads / 8 cores). Each core
receives the full (transposed) q/k/v plus its head's weight slices, computes
its head end-to-end including the rank-64 slice of the output projection, and
the host sums the 8 partial outputs.

v2 layout/engine plan:
  - weights packed into one bf16 blob + one f32 blob, DMA'd first
  - k/q projections emit [128,*] dup outputs (lhsT = [W|W]) so P2 can run
    kb-pairs concurrently in row groups {0-63} / {64-127}
  - fenmu = sum_kb exp(.) via bf16 identity-matmul PSUM accumulation
  - softmax/LN stats via grouped bn_stats (even/odd combine) + 1-NR rsqrt
  - w0 applied pre-transpose as a per-partition scalar (no rank-1 matmuls)
  - P6 single K=128 matmul per chunk: lhsT = [Wo;4Wo], rhs = [heads;qh]
  - output DRAM layout [B,128,4,S] (4KB/partition descriptors)
"""

import numpy as np

BS, S, DM, H, DK = 4, 512, 512, 8, 64
NCORES = 8
WCOLS = 512 + 512 + 256 + 512 + 64 + 128  # Wk,Wq,Wv,WoB,bvb,id = 1984
FCOLS = 8 + 128  # biases + idf


def build_program(nc, tile, mybir):
    f32 = mybir.dt.float32
    bf16 = mybir.dt.bfloat16
    i32 = mybir.dt.int32
    AF = mybir.ActivationFunctionType
    OP = mybir.AluOpType

    wb_d = nc.dram_tensor("wblob", [128, WCOLS], bf16, kind="ExternalInput")
    fb_d = nc.dram_tensor("fblob", [128, FCOLS], f32, kind="ExternalInput")
    kT_d = nc.dram_tensor("kT", [BS, 128, 4, S], bf16, kind="ExternalInput")
    qT_d = nc.dram_tensor("qT", [BS, 128, 4, S], bf16, kind="ExternalInput")
    vT_d = nc.dram_tensor("vT", [BS, 128, 4, S], bf16, kind="ExternalInput")
    outT_d = nc.dram_tensor("outT", [BS, 128, 4, S], bf16, kind="ExternalOutput")

    with tile.TileContext(nc) as tc:
        with (
            tc.tile_pool(name="persist", bufs=1) as pp,
            tc.tile_pool(name="inp", bufs=3) as inp,
            tc.tile_pool(name="ex", bufs=2) as exp_,
            tc.tile_pool(name="wk", bufs=2) as wkp,
            tc.tile_pool(name="st", bufs=1) as stp,
            tc.tile_pool(name="ob", bufs=2) as obp,
            tc.tile_pool(name="psum", bufs=1, space="PSUM") as psp,
        ):
            # ---- persistent SBUF ----
            wb = pp.tile([128, WCOLS], bf16, tag="wb")
            fb = pp.tile([128, FCOLS], f32, tag="fb")
            Wk_s = wb[:, 0:512].rearrange("p (m c) -> p m c", m=4)
            Wq_s = wb[:, 512:1024].rearrange("p (m c) -> p m c", m=4)
            Wv_s = wb[:, 1024:1280].rearrange("p (m c) -> p m c", m=4)
            WoB_s = wb[:, 1280:1792].rearrange("p (m c) -> p m c", m=4)
            bvb_s = wb[:, 1792:1856]
            id_s = wb[:, 1856:1984]
            bk2 = fb[:, 0:1]
            bq2 = fb[:, 1:2]
            bo4 = fb[:, 2:6]
            alx = fb[:, 6:7]
            b4x = fb[:, 7:8]
            idf_s = fb[:, 8:136]

            khT2 = pp.tile([128, BS, S], bf16, tag="khT2")
            qhT2 = pp.tile([128, BS, S], bf16, tag="qhT2")
            vh_all = pp.tile([128, 4, BS, DK], bf16, tag="vh")  # [j,jc,c,d]
            heads = pp.tile([128, BS, S], bf16, tag="heads")  # 0:64 ln, 64: qh
            Z_all = pp.tile([128, BS, 16], f32, tag="Z")   # (b, ic*4+c)
            Q_all = pp.tile([128, BS, 16], f32, tag="Q")
            w1_all = pp.tile([128, BS, 16], f32, tag="w1")
            w0_all = pp.tile([128, 16], bf16, tag="w0")
            wup = pp.tile([1, 8], f32, tag="wup")

            def psum_pe():
                return psp.tile([128, 2, S], f32, tag="pe", bufs=2, name="ppe")

            def psum_fs(shape):
                return psp.tile(shape, f32, tag="fs", bufs=2, name="pfs")

            def psum_po(shape):
                return psp.tile(shape, f32, tag="po", bufs=2, name="ppo")

            # ---- prelude: ACT table preload + DMAs in use-order ----
            nc.vector.memset(wup[:], 1.0)
            nc.scalar.activation(wup[:], wup[:], AF.Exp)
            nc.sync.dma_start(wb[:], wb_d[:])
            ktiles = [inp.tile([128, 4, S], bf16, tag="kt", bufs=4,
                               name=f"kt{b}") for b in range(BS)]
            qtiles = [inp.tile([128, 4, S], bf16, tag="qt", bufs=4,
                               name=f"qt{b}") for b in range(BS)]
            vtiles = [inp.tile([128, 4, S], bf16, tag="vt", bufs=4,
                               name=f"vt{b}") for b in range(BS)]
            # trigger order = sync-engine issue order: wblob first, then k
            # (P2 needs all of kh), q0, then v and the remaining q's.
            nc.sync.dma_start(ktiles[0][:], kT_d[0])
            nc.sync.dma_start(ktiles[1][:], kT_d[1])
            nc.sync.dma_start(qtiles[0][:], qT_d[0])
            nc.sync.dma_start(ktiles[2][:], kT_d[2])
            nc.sync.dma_start(ktiles[3][:], kT_d[3])
            nc.sync.dma_start(qtiles[1][:], qT_d[1])
            nc.sync.dma_start(fb[:], fb_d[:])
            nc.sync.dma_start(qtiles[2][:], qT_d[2])
            nc.sync.dma_start(qtiles[3][:], qT_d[3])
            nc.sync.dma_start(vtiles[0][:], vT_d[0])
            nc.sync.dma_start(vtiles[1][:], vT_d[1])
            nc.sync.dma_start(vtiles[2][:], vT_d[2])
            nc.sync.dma_start(vtiles[3][:], vT_d[3])

            # ---- phase emitters ----
            def emit_p1(W_s, bias_c, src, dst, b):
                ps = psum_po([128, S])
                for mc in range(4):
                    nc.tensor.matmul(ps[:], W_s[:, mc, :], src[:, mc, :],
                                     start=(mc == 0), stop=(mc == 3))
                nc.vector.tensor_scalar(dst[:, b, :], ps[:], bias_c, None,
                                        op0=OP.add)

            def emit_hq(b):
                nc.vector.tensor_copy(heads[64:128, b, :], qhT2[64:128, b, :])

            def emit_vh(c):
                pv = psum_po([128, 4, DK])
                vt = vtiles[c]
                for jc in range(4):
                    for mc in range(4):
                        nc.tensor.matmul(
                            pv[:, jc, :], vt[:, mc, jc * 128:(jc + 1) * 128],
                            Wv_s[:, mc, :], start=(mc == 0), stop=(mc == 3))
                nc.vector.tensor_tensor(
                    vh_all[:, :, c, :], pv[:],
                    bvb_s[:].unsqueeze(1).broadcast_to((128, 4, DK)),
                    op=OP.add)

            rtiles = {}

            extiles = {}

            def emit_p2h(b, jc, half):
                jcs = slice(jc * 128, (jc + 1) * 128)
                if half == 0:
                    extiles[(b, jc)] = exp_.tile([128, 4, S], bf16, tag="ex",
                                                 bufs=4, name="exq")
                exq = extiles[(b, jc)]
                ph = psum_pe()
                kb0, kb1 = 2 * half, 2 * half + 1
                nc.tensor.matmul(ph[:, 0, :], khT2[0:64, kb0, jcs],
                                 qhT2[0:64, b, :], start=True, stop=True)
                nc.tensor.matmul(ph[:, 1, :], khT2[64:128, kb1, jcs],
                                 qhT2[64:128, b, :], start=True, stop=True)
                nc.scalar.activation(
                    exq[:, 2 * half:2 * half + 2, :], ph[:], AF.Exp)

            def emit_p2f(b, jc):
                exq = extiles[(b, jc)]
                fp = psum_fs([128, S])
                for kb in range(4):
                    nc.tensor.matmul(fp[:], id_s[:], exq[:, kb, :],
                                     start=(kb == 0), stop=(kb == 3))
                wrec = wkp.tile([128, S], f32, tag="wrec", bufs=3, name="wrec")
                nc.vector.reciprocal_approx_fast(wrec[:], fp[:])
                if jc == 0:
                    rtiles[b] = wkp.tile([128, 4, S], bf16, tag="rt",
                                         bufs=4, name=f"rt{b}")
                nc.gpsimd.tensor_tensor(rtiles[b][:, jc, :], exq[:, b, :],
                                        wrec[:], op=OP.mult)

            def emit_p2(b, jc):
                emit_p2h(b, jc, 0)
                emit_p2h(b, jc, 1)
                emit_p2f(b, jc)

            etiles = {}

            def emit_p3(b, p):
                rt = rtiles[b]
                if p == 0:
                    etiles[b] = wkp.tile([128, 4, BS, DK], bf16, tag="ea",
                                         bufs=4, name=f"ea{b}")
                ea = etiles[b]
                scp = psum_fs([128, 2, BS * DK])
                for u in range(2):
                    ic = 2 * p + u
                    ics = slice(ic * 128, (ic + 1) * 128)
                    for jc in range(4):
                        nc.tensor.matmul(
                            scp[:, u, :], rt[:, jc, ics],
                            vh_all[:, jc].rearrange("p c d -> p (c d)"),
                            start=(jc == 0), stop=(jc == 3))
                nc.scalar.activation(
                    ea[:, 2 * p:2 * p + 2].rearrange("p a c d -> p a (c d)"),
                    scp[:], AF.Exp)

            def emit_zq(b, eng):
                # e2 = e*e (gpsimd), then grouped reduces -> Z, Q (DVE-only)
                ea = etiles[b]
                e2 = wkp.tile([128, 16, DK], bf16, tag="e2", name="e2")
                eav = ea[:].rearrange("p a c d -> p (a c) d")
                eng.tensor_tensor(e2[:], eav, eav, op=OP.mult)
                nc.vector.tensor_reduce(Z_all[:, b, :], eav,
                                        axis=mybir.AxisListType.X, op=OP.add)
                nc.vector.tensor_reduce(Q_all[:, b, :], e2[:],
                                        axis=mybir.AxisListType.X, op=OP.add)

            SQ63 = float(np.sqrt(63.0))

            def emit_stats(b0, nb):
                # batches [b0, b0+nb): ln scale-invariance kills 1/Z:
                # w1 = sqrt(63)*rsqrt(Q - Z^2/64), w0 = -Z*w1/64
                bsl = slice(b0, b0 + nb)
                cnt = [128, 16 * nb]
                Zv = Z_all[:, bsl, :].rearrange("p b g -> p (b g)")
                Qv = Q_all[:, bsl, :].rearrange("p b g -> p (b g)")
                t_ = stp.tile(cnt, f32, tag=f"t{b0}", name="t_")
                nc.vector.tensor_tensor(t_[:], Zv, Zv, op=OP.mult)
                s_ = stp.tile(cnt, f32, tag=f"s{b0}", name="s_")
                nc.vector.scalar_tensor_tensor(s_[:], t_[:], -1.0 / DK, Qv,
                                               op0=OP.mult, op1=OP.add)
                # rsqrt seed + 1 NR iter (w1 fused into the final mult)
                r_ = stp.tile(cnt, f32, tag=f"r{b0}", name="r_")
                nc.vector.tensor_scalar(r_[:].bitcast(i32), s_[:].bitcast(i32),
                                        1, None, op0=OP.logical_shift_right)
                nc.vector.tensor_scalar(r_[:].bitcast(i32), r_[:].bitcast(i32),
                                        -1, 0x5F3759DF, op0=OP.mult, op1=OP.add)
                nt = stp.tile(cnt, f32, tag=f"n{b0}", name="nt")
                nc.vector.tensor_tensor(nt[:], s_[:], r_[:], op=OP.mult)
                nc.vector.tensor_tensor(nt[:], nt[:], r_[:], op=OP.mult)
                nc.vector.tensor_scalar(nt[:], nt[:], -0.5, 1.5,
                                        op0=OP.mult, op1=OP.add)
                w1v = w1_all[:, bsl, :].rearrange("p b g -> p (b g)")
                nc.vector.scalar_tensor_tensor(w1v, r_[:], SQ63, nt[:],
                                               op0=OP.mult, op1=OP.mult)
                zg = stp.tile(cnt, f32, tag=f"zg{b0}", name="zg")
                nc.vector.scalar_tensor_tensor(zg[:], Zv, -1.0 / DK, w1v,
                                               op0=OP.mult, op1=OP.mult)
                w0r = stp.tile([128, 4 * nb], f32, tag=f"w{b0}", name="w0r")
                nc.vector.tensor_reduce(
                    w0r[:], zg[:].rearrange("p (g c) -> p g c", c=4),
                    axis=mybir.AxisListType.X, op=OP.add)
                nc.vector.tensor_copy(w0_all[:, 4 * b0:4 * (b0 + nb)], w0r[:])

            bsctiles = {}

            def emit_bsc(b, eng):
                ea = etiles[b]
                bsc = obp.tile([128, 16, DK], bf16, tag="bsc", bufs=4,
                               name=f"bsc{b}")
                bsctiles[b] = bsc
                w1b = (w1_all[:, b, :].unsqueeze(-1)
                       .broadcast_to((128, 16, DK)))
                eng.tensor_tensor(
                    bsc[:], ea[:].rearrange("p a c d -> p (a c) d"), w1b,
                    op=OP.mult)

            def emit_p5(b):
                bsc = bsctiles[b]
                bp = psum_fs([128, 4, DK])
                for ic in range(4):
                    for c in range(4):
                        nc.tensor.matmul(bp[:, ic, :], id_s[:],
                                         bsc[:, ic * 4 + c, :],
                                         start=(c == 0), stop=False)
                    # += w0 broadcast along d via a 0-stride rhs
                    nc.tensor.matmul(
                        bp[:, ic, :], id_s[:],
                        w0_all[:, b * 4 + ic:b * 4 + ic + 1]
                        .broadcast_to((128, DK)),
                        start=False, stop=True)
                balls = obp.tile([128, 4, DK], f32, tag="balls", name="balls")
                nc.vector.tensor_copy(balls[:], bp[:])
                pt = psum_po([64, S])
                for ic in range(4):
                    nc.tensor.matmul(pt[0:64, ic * 128:(ic + 1) * 128],
                                     balls[:, ic, :], idf_s,
                                     is_transpose=True, start=True, stop=True)
                nc.scalar.activation(heads[0:64, b, :], pt[0:64, :],
                                     AF.Identity, bias=b4x[0:64, :],
                                     scale=alx[0:64, :])

            def emit_p6(b):
                osb = obp.tile([128, 4, S], bf16, tag="osb", name="osb")
                for nch in range(4):
                    pp6 = psum_po([128, S])
                    nc.tensor.matmul(pp6[:], WoB_s[:, nch, :], heads[:, b, :],
                                     start=True, stop=True)
                    if nch % 2 == 0:
                        nc.scalar.activation(osb[:, nch, :], pp6[:],
                                             AF.Identity,
                                             bias=bo4[:, nch:nch + 1])
                    else:
                        nc.vector.tensor_scalar(osb[:, nch, :], pp6[:],
                                                bo4[:, nch:nch + 1], None,
                                                op0=OP.add)
                nc.sync.dma_start(outT_d[b], osb[:])

            # ---- emission schedule (engine queues are FIFO: never emit
            # work whose inputs arrive later than the next emission's) ----
            emit_p1(Wk_s, bk2, ktiles[0], khT2, 0)
            emit_p1(Wk_s, bk2, ktiles[1], khT2, 1)
            emit_p1(Wq_s, bq2, qtiles[0], qhT2, 0)
            emit_hq(0)
            # pair-A of b=0 needs only kh[0:2]: start ACT early
            emit_p2h(0, 0, 0)
            emit_p2h(0, 1, 0)
            emit_p1(Wk_s, bk2, ktiles[2], khT2, 2)
            emit_p2h(0, 2, 0)
            emit_p1(Wk_s, bk2, ktiles[3], khT2, 3)
            emit_p2h(0, 3, 0)
            emit_p2h(0, 0, 1)
            emit_p2f(0, 0)
            emit_p1(Wq_s, bq2, qtiles[1], qhT2, 1)
            emit_hq(1)
            emit_p2h(0, 1, 1)
            emit_p2f(0, 1)
            emit_p2h(0, 2, 1)
            emit_p2f(0, 2)
            emit_p1(Wq_s, bq2, qtiles[2], qhT2, 2)
            emit_hq(2)
            emit_p2h(0, 3, 1)
            emit_p2f(0, 3)
            emit_p2(1, 0)
            emit_p1(Wq_s, bq2, qtiles[3], qhT2, 3)
            emit_hq(3)
            emit_p2(1, 1)
            emit_p2(1, 2)
            emit_p2(1, 3)
            emit_vh(0)
            emit_p2(2, 0)
            emit_vh(1)
            emit_p2(2, 1)
            emit_vh(2)
            emit_p2(2, 2)
            emit_vh(3)
            emit_p2(2, 3)
            emit_p2(3, 0)
            emit_p3(0, 0)
            emit_p2(3, 1)
            emit_p3(0, 1)
            emit_p2(3, 2)
            emit_zq(0, nc.vector)
            emit_stats(0, 1)
            emit_bsc(0, nc.gpsimd)
            emit_p3(1, 0)
            emit_p2(3, 3)
            emit_p3(1, 1)
            emit_zq(1, nc.vector)
            emit_stats(1, 1)
            emit_bsc(1, nc.gpsimd)
            emit_p3(2, 0)
            emit_p3(2, 1)
            emit_zq(2, nc.vector)
            emit_stats(2, 1)
            emit_bsc(2, nc.gpsimd)
            emit_p3(3, 0)
            emit_p3(3, 1)
            emit_zq(3, nc.vector)
            emit_stats(3, 1)
            emit_bsc(3, nc.vector)
            emit_p5(0)
            emit_p6(0)
            emit_p5(1)
            emit_p6(1)
            emit_p5(2)
            emit_p6(2)
            emit_p5(3)
            emit_p6(3)

    return nc


def _build():
    import concourse.bass as bass  # noqa
    import concourse.tile as tile
    from concourse import bacc, mybir

    nc = bacc.Bacc("TRN2", target_bir_lowering=False, debug=False,
                   num_devices=NCORES)
    build_program(nc, tile, mybir)
    nc.compile()
    return nc


_cached_nc = None


def make_in_maps(q, k, v, Wq, bq, Wk, bk, Wv, bv, Wo, bo, alpha, beta):
    import ml_dtypes
    bft = ml_dtypes.bfloat16

    def prelay(x):
        xT = np.swapaxes(np.asarray(x, np.float32), 1, 2)  # [B, DM, S]
        return np.ascontiguousarray(
            xT.reshape(BS, 4, 128, S).transpose(0, 2, 1, 3)).astype(bft)

    def wlay(W):  # [DM, DK] -> [128, 4, DK]
        return np.ascontiguousarray(
            np.asarray(W, np.float32).reshape(4, 128, DK).transpose(1, 0, 2))

    qT, kT, vT = prelay(q), prelay(k), prelay(v)
    Wq, Wk, Wv, Wo = (np.asarray(x, np.float32) for x in (Wq, Wk, Wv, Wo))
    bq, bk, bv, bo = (np.asarray(x, np.float32) for x in (bq, bk, bv, bo))
    alpha, beta = np.asarray(alpha, np.float32), np.asarray(beta, np.float32)
    scale = np.float32(1.0 / np.sqrt(np.float32(DK)))
    idbf = np.eye(128, dtype=np.float32)
    in_maps = []
    for h in range(NCORES):
        sl = slice(h * DK, (h + 1) * DK)
        WkD = wlay(Wk[:, sl])
        WkD = np.concatenate([WkD, WkD], axis=2).reshape(128, 512)
        WqD = wlay(Wq[:, sl])
        WqD = np.concatenate([WqD, WqD], axis=2).reshape(128, 512)
        WvS = wlay(Wv[:, sl] * scale).reshape(128, 256)
        WoStack = np.concatenate([Wo[sl, :], 4.0 * Wo[sl, :]], axis=0)
        WoB = WoStack.reshape(128, 4, 128).reshape(128, 512)
        bvb = np.tile((bv[sl] * scale)[None, :], (128, 1))
        wblob = np.ascontiguousarray(np.concatenate(
            [WkD, WqD, WvS, WoB, bvb, idbf], axis=1)).astype(bft)
        bo_h = bo if h == 0 else np.zeros_like(bo)
        alx = np.zeros(128, np.float32)
        alx[0:DK] = alpha
        b4x = np.zeros(128, np.float32)
        b4x[0:DK] = 4.0 * beta
        fblob = np.ascontiguousarray(np.concatenate(
            [np.tile(bk[sl], 2)[:, None], np.tile(bq[sl], 2)[:, None],
             bo_h.reshape(4, 128).T, alx[:, None], b4x[:, None], idbf],
            axis=1)).astype(np.float32)
        in_maps.append({
            "wblob": wblob, "fblob": fblob,
            "kT": kT, "qT": qT, "vT": vT,
        })
    return in_maps


def assemble(results):
    out = np.zeros((BS, S, DM), np.float32)
    for r in results:
        out += np.asarray(r["outT"], np.float32).transpose(0, 3, 2, 1).reshape(
            BS, S, DM)
    return out


def kernel(**inputs) -> np.ndarray:
    global _cached_nc
    from concourse.bass_utils import run_bass_kernel_spmd

    if _cached_nc is None:
        _cached_nc = _build()
    in_maps = make_in_maps(**inputs)
    res = run_bass_kernel_spmd(_cached_nc, in_maps, list(range(NCORES)))
    return assemble(res.results)


# revision 24
# speedup vs baseline: 1.0703x; 1.0078x over previous
"""Trainium2 Bass kernel for nn_MultiHeadCrossAttention (BS=4, S=512, DM=512, H=8).

Sharding: one attention head per NeuronCore (8 heads / 8 cores). Each core
receives the full (transposed) q/k/v plus its head's weight slices, computes
its head end-to-end including the rank-64 slice of the output projection, and
the host sums the 8 partial outputs.

v2 layout/engine plan:
  - weights packed into one bf16 blob + one f32 blob, DMA'd first
  - k/q projections emit [128,*] dup outputs (lhsT = [W|W]) so P2 can run
    kb-pairs concurrently in row groups {0-63} / {64-127}
  - fenmu = sum_kb exp(.) via bf16 identity-matmul PSUM accumulation
  - softmax/LN stats via grouped bn_stats (even/odd combine) + 1-NR rsqrt
  - w0 applied pre-transpose as a per-partition scalar (no rank-1 matmuls)
  - P6 single K=128 matmul per chunk: lhsT = [Wo;4Wo], rhs = [heads;qh]
  - output DRAM layout [B,128,4,S] (4KB/partition descriptors)
"""

import numpy as np

BS, S, DM, H, DK = 4, 512, 512, 8, 64
NCORES = 8
WCOLS = 512 + 512 + 256 + 512 + 64 + 128  # Wk,Wq,Wv,WoB,bvb,id = 1984
FCOLS = 8 + 128  # biases + idf


def build_program(nc, tile, mybir):
    f32 = mybir.dt.float32
    bf16 = mybir.dt.bfloat16
    i32 = mybir.dt.int32
    AF = mybir.ActivationFunctionType
    OP = mybir.AluOpType

    wb_d = nc.dram_tensor("wblob", [128, WCOLS], bf16, kind="ExternalInput")
    fb_d = nc.dram_tensor("fblob", [128, FCOLS], f32, kind="ExternalInput")
    kT_d = nc.dram_tensor("kT", [BS, 128, 4, S], bf16, kind="ExternalInput")
    qT_d = nc.dram_tensor("qT", [BS, 128, 4, S], bf16, kind="ExternalInput")
    vT_d = nc.dram_tensor("vT", [BS, 128, 4, S], bf16, kind="ExternalInput")
    outT_d = nc.dram_tensor("outT", [BS, 128, 4, S], bf16, kind="ExternalOutput")

    with tile.TileContext(nc) as tc:
        with (
            tc.tile_pool(name="persist", bufs=1) as pp,
            tc.tile_pool(name="inp", bufs=3) as inp,
            tc.tile_pool(name="ex", bufs=2) as exp_,
            tc.tile_pool(name="wk", bufs=2) as wkp,
            tc.tile_pool(name="st", bufs=1) as stp,
            tc.tile_pool(name="ob", bufs=2) as obp,
            tc.tile_pool(name="psum", bufs=1, space="PSUM") as psp,
        ):
            # ---- persistent SBUF ----
            wb = pp.tile([128, WCOLS], bf16, tag="wb")
            fb = pp.tile([128, FCOLS], f32, tag="fb")
            Wk_s = wb[:, 0:512].rearrange("p (m c) -> p m c", m=4)
            Wq_s = wb[:, 512:1024].rearrange("p (m c) -> p m c", m=4)
            Wv_s = wb[:, 1024:1280].rearrange("p (m c) -> p m c", m=4)
            WoB_s = wb[:, 1280:1792].rearrange("p (m c) -> p m c", m=4)
            bvb_s = wb[:, 1792:1856]
            id_s = wb[:, 1856:1984]
            bk2 = fb[:, 0:1]
            bq2 = fb[:, 1:2]
            bo4 = fb[:, 2:6]
            alx = fb[:, 6:7]
            b4x = fb[:, 7:8]
            idf_s = fb[:, 8:136]

            khT2 = pp.tile([128, BS, S], bf16, tag="khT2")
            qhT2 = pp.tile([128, BS, S], bf16, tag="qhT2")
            vh_all = pp.tile([128, 4, BS, DK], bf16, tag="vh")  # [j,jc,c,d]
            heads = pp.tile([128, BS, S], bf16, tag="heads")  # 0:64 ln, 64: qh
            Z_all = pp.tile([128, BS, 16], f32, tag="Z")   # (b, ic*4+c)
            Q_all = pp.tile([128, BS, 16], f32, tag="Q")
            w1_all = pp.tile([128, BS, 16], f32, tag="w1")
            w0_all = pp.tile([128, 16], bf16, tag="w0")
            wup = pp.tile([1, 8], f32, tag="wup")

            def psum_pe():
                return psp.tile([128, 2, S], f32, tag="pe", bufs=2, name="ppe")

            def psum_fs(shape):
                return psp.tile(shape, f32, tag="fs", bufs=2, name="pfs")

            def psum_po(shape):
                return psp.tile(shape, f32, tag="po", bufs=2, name="ppo")

            # ---- prelude: ACT table preload + DMAs in use-order ----
            nc.vector.memset(wup[:], 1.0)
            nc.scalar.activation(wup[:], wup[:], AF.Exp)
            nc.sync.dma_start(wb[:], wb_d[:])
            nc.sync.dma_start(fb[:], fb_d[:])
            ktiles = [inp.tile([128, 4, S], bf16, tag="kt", bufs=4,
                               name=f"kt{b}") for b in range(BS)]
            qtiles = [inp.tile([128, 4, S], bf16, tag="qt", bufs=4,
                               name=f"qt{b}") for b in range(BS)]
            vtiles = [inp.tile([128, 4, S], bf16, tag="vt", bufs=4,
                               name=f"vt{b}") for b in range(BS)]
            # trigger order = sync-engine issue order: wblob first, then k
            # (P2 needs all of kh), q0, then v and the remaining q's.
            nc.sync.dma_start(ktiles[0][:], kT_d[0])
            nc.sync.dma_start(ktiles[1][:], kT_d[1])
            nc.sync.dma_start(qtiles[0][:], qT_d[0])
            nc.sync.dma_start(ktiles[2][:], kT_d[2])
            nc.sync.dma_start(ktiles[3][:], kT_d[3])
            nc.sync.dma_start(qtiles[1][:], qT_d[1])
            nc.sync.dma_start(qtiles[2][:], qT_d[2])
            nc.sync.dma_start(qtiles[3][:], qT_d[3])
            nc.sync.dma_start(vtiles[0][:], vT_d[0])
            nc.sync.dma_start(vtiles[1][:], vT_d[1])
            nc.sync.dma_start(vtiles[2][:], vT_d[2])
            nc.sync.dma_start(vtiles[3][:], vT_d[3])

            # ---- phase emitters ----
            def emit_p1(W_s, bias_c, src, dst, b, alt=False):
                ps = (psp.tile([128, S], f32, tag="pe", bufs=2, name="ppe")
                      if alt else psum_po([128, S]))
                for mc in range(4):
                    nc.tensor.matmul(ps[:], W_s[:, mc, :], src[:, mc, :],
                                     start=(mc == 0), stop=(mc == 3))
                nc.vector.tensor_scalar(dst[:, b, :], ps[:], bias_c, None,
                                        op0=OP.add)

            def emit_hq(b):
                nc.vector.tensor_copy(heads[64:128, b, :], qhT2[64:128, b, :])

            def emit_vh(c):
                pv = psum_po([128, 4, DK])
                vt = vtiles[c]
                for jc in range(4):
                    for mc in range(4):
                        nc.tensor.matmul(
                            pv[:, jc, :], vt[:, mc, jc * 128:(jc + 1) * 128],
                            Wv_s[:, mc, :], start=(mc == 0), stop=(mc == 3))
                nc.vector.tensor_tensor(
                    vh_all[:, :, c, :], pv[:],
                    bvb_s[:].unsqueeze(1).broadcast_to((128, 4, DK)),
                    op=OP.add)

            rtiles = {}

            extiles = {}

            def emit_p2h(b, jc, half):
                jcs = slice(jc * 128, (jc + 1) * 128)
                if half == 0:
                    extiles[(b, jc)] = exp_.tile([128, 4, S], bf16, tag="ex",
                                                 bufs=4, name="exq")
                exq = extiles[(b, jc)]
                ph = psum_pe()
                kb0, kb1 = 2 * half, 2 * half + 1
                nc.tensor.matmul(ph[:, 0, :], khT2[0:64, kb0, jcs],
                                 qhT2[0:64, b, :], start=True, stop=True)
                nc.tensor.matmul(ph[:, 1, :], khT2[64:128, kb1, jcs],
                                 qhT2[64:128, b, :], start=True, stop=True)
                nc.scalar.activation(
                    exq[:, 2 * half:2 * half + 2, :], ph[:], AF.Exp)

            def emit_p2f(b, jc):
                exq = extiles[(b, jc)]
                fp = psum_fs([128, S])
                for kb in range(4):
                    nc.tensor.matmul(fp[:], id_s[:], exq[:, kb, :],
                                     start=(kb == 0), stop=(kb == 3))
                wrec = wkp.tile([128, S], f32, tag="wrec", bufs=3, name="wrec")
                nc.vector.reciprocal_approx_fast(wrec[:], fp[:])
                if jc == 0:
                    rtiles[b] = wkp.tile([128, 4, S], bf16, tag="rt",
                                         bufs=4, name=f"rt{b}")
                nc.gpsimd.tensor_tensor(rtiles[b][:, jc, :], exq[:, b, :],
                                        wrec[:], op=OP.mult)

            def emit_p2(b, jc):
                emit_p2h(b, jc, 0)
                emit_p2h(b, jc, 1)
                emit_p2f(b, jc)

            etiles = {}

            def emit_p3(b, p):
                rt = rtiles[b]
                if p == 0:
                    etiles[b] = wkp.tile([128, 4, BS, DK], bf16, tag="ea",
                                         bufs=4, name=f"ea{b}")
                ea = etiles[b]
                scp = psum_fs([128, 2, BS * DK])
                for u in range(2):
                    ic = 2 * p + u
                    ics = slice(ic * 128, (ic + 1) * 128)
                    for jc in range(4):
                        nc.tensor.matmul(
                            scp[:, u, :], rt[:, jc, ics],
                            vh_all[:, jc].rearrange("p c d -> p (c d)"),
                            start=(jc == 0), stop=(jc == 3))
                nc.scalar.activation(
                    ea[:, 2 * p:2 * p + 2].rearrange("p a c d -> p a (c d)"),
                    scp[:], AF.Exp)

            def emit_zq(b, eng):
                # e2 = e*e (gpsimd), then grouped reduces -> Z, Q (DVE-only)
                ea = etiles[b]
                e2 = wkp.tile([128, 16, DK], bf16, tag="e2", name="e2")
                eav = ea[:].rearrange("p a c d -> p (a c) d")
                eng.tensor_tensor(e2[:], eav, eav, op=OP.mult)
                nc.vector.tensor_reduce(Z_all[:, b, :], eav,
                                        axis=mybir.AxisListType.X, op=OP.add)
                nc.vector.tensor_reduce(Q_all[:, b, :], e2[:],
                                        axis=mybir.AxisListType.X, op=OP.add)

            SQ63 = float(np.sqrt(63.0))

            def emit_stats(b0, nb):
                # batches [b0, b0+nb): ln scale-invariance kills 1/Z:
                # w1 = sqrt(63)*rsqrt(Q - Z^2/64), w0 = -Z*w1/64
                bsl = slice(b0, b0 + nb)
                cnt = [128, 16 * nb]
                Zv = Z_all[:, bsl, :].rearrange("p b g -> p (b g)")
                Qv = Q_all[:, bsl, :].rearrange("p b g -> p (b g)")
                t_ = stp.tile(cnt, f32, tag=f"t{b0}", name="t_")
                nc.vector.tensor_tensor(t_[:], Zv, Zv, op=OP.mult)
                s_ = stp.tile(cnt, f32, tag=f"s{b0}", name="s_")
                nc.vector.scalar_tensor_tensor(s_[:], t_[:], -1.0 / DK, Qv,
                                               op0=OP.mult, op1=OP.add)
                # rsqrt seed + 1 NR iter (w1 fused into the final mult)
                r_ = stp.tile(cnt, f32, tag=f"r{b0}", name="r_")
                nc.vector.tensor_scalar(r_[:].bitcast(i32), s_[:].bitcast(i32),
                                        1, None, op0=OP.logical_shift_right)
                nc.vector.tensor_scalar(r_[:].bitcast(i32), r_[:].bitcast(i32),
                                        -1, 0x5F3759DF, op0=OP.mult, op1=OP.add)
                nt = stp.tile(cnt, f32, tag=f"n{b0}", name="nt")
                nc.vector.tensor_tensor(nt[:], s_[:], r_[:], op=OP.mult)
                nc.vector.tensor_tensor(nt[:], nt[:], r_[:], op=OP.mult)
                nc.vector.tensor_scalar(nt[:], nt[:], -0.5, 1.5,
                                        op0=OP.mult, op1=OP.add)
                w1v = w1_all[:, bsl, :].rearrange("p b g -> p (b g)")
                nc.vector.scalar_tensor_tensor(w1v, r_[:], SQ63, nt[:],
                                               op0=OP.mult, op1=OP.mult)
                zg = stp.tile(cnt, f32, tag=f"zg{b0}", name="zg")
                nc.vector.scalar_tensor_tensor(zg[:], Zv, -1.0 / DK, w1v,
                                               op0=OP.mult, op1=OP.mult)
                w0r = stp.tile([128, 4 * nb], f32, tag=f"w{b0}", name="w0r")
                nc.vector.tensor_reduce(
                    w0r[:], zg[:].rearrange("p (g c) -> p g c", c=4),
                    axis=mybir.AxisListType.X, op=OP.add)
                nc.vector.tensor_copy(w0_all[:, 4 * b0:4 * (b0 + nb)], w0r[:])

            bsctiles = {}

            def emit_bsc(b, eng):
                ea = etiles[b]
                bsc = obp.tile([128, 16, DK], bf16, tag="bsc", bufs=4,
                               name=f"bsc{b}")
                bsctiles[b] = bsc
                w1b = (w1_all[:, b, :].unsqueeze(-1)
                       .broadcast_to((128, 16, DK)))
                eng.tensor_tensor(
                    bsc[:], ea[:].rearrange("p a c d -> p (a c) d"), w1b,
                    op=OP.mult)

            def emit_p5(b):
                bsc = bsctiles[b]
                bp = psum_fs([128, 4, DK])
                for ic in range(4):
                    for c in range(4):
                        nc.tensor.matmul(bp[:, ic, :], id_s[:],
                                         bsc[:, ic * 4 + c, :],
                                         start=(c == 0), stop=False)
                    # += w0 broadcast along d via a 0-stride rhs
                    nc.tensor.matmul(
                        bp[:, ic, :], id_s[:],
                        w0_all[:, b * 4 + ic:b * 4 + ic + 1]
                        .broadcast_to((128, DK)),
                        start=False, stop=True)
                balls = obp.tile([128, 4, DK], f32, tag="balls", name="balls")
                nc.vector.tensor_copy(balls[:], bp[:])
                pt = psum_po([64, S])
                for ic in range(4):
                    nc.tensor.matmul(pt[0:64, ic * 128:(ic + 1) * 128],
                                     balls[:, ic, :], idf_s,
                                     is_transpose=True, start=True, stop=True)
                nc.scalar.activation(heads[0:64, b, :], pt[0:64, :],
                                     AF.Identity, bias=b4x[0:64, :],
                                     scale=alx[0:64, :])

            def emit_p6(b):
                osb = obp.tile([128, 4, S], bf16, tag="osb", name="osb")
                for nch in range(4):
                    pp6 = psum_po([128, S])
                    nc.tensor.matmul(pp6[:], WoB_s[:, nch, :], heads[:, b, :],
                                     start=True, stop=True)
                    if nch % 2 == 0:
                        nc.scalar.activation(osb[:, nch, :], pp6[:],
                                             AF.Identity,
                                             bias=bo4[:, nch:nch + 1])
                    else:
                        nc.vector.tensor_scalar(osb[:, nch, :], pp6[:],
                                                bo4[:, nch:nch + 1], None,
                                                op0=OP.add)
                nc.sync.dma_start(outT_d[b], osb[:])

            # ---- emission schedule (engine queues are FIFO: never emit
            # work whose inputs arrive later than the next emission's) ----
            emit_p1(Wk_s, bk2, ktiles[0], khT2, 0)
            emit_p1(Wk_s, bk2, ktiles[1], khT2, 1, alt=True)
            emit_p1(Wq_s, bq2, qtiles[0], qhT2, 0)
            emit_hq(0)
            # pair-A of b=0 needs only kh[0:2]: start ACT early
            emit_p2h(0, 0, 0)
            emit_p2h(0, 1, 0)
            emit_p1(Wk_s, bk2, ktiles[2], khT2, 2, alt=True)
            emit_p2h(0, 2, 0)
            emit_p1(Wk_s, bk2, ktiles[3], khT2, 3)
            emit_p2h(0, 3, 0)
            emit_p2h(0, 0, 1)
            emit_p2f(0, 0)
            emit_p1(Wq_s, bq2, qtiles[1], qhT2, 1)
            emit_hq(1)
            emit_p2h(0, 1, 1)
            emit_p2f(0, 1)
            emit_p2h(0, 2, 1)
            emit_p2f(0, 2)
            emit_p1(Wq_s, bq2, qtiles[2], qhT2, 2)
            emit_hq(2)
            emit_p2h(0, 3, 1)
            emit_p2f(0, 3)
            emit_p2(1, 0)
            emit_p1(Wq_s, bq2, qtiles[3], qhT2, 3)
            emit_hq(3)
            emit_p2(1, 1)
            emit_p2(1, 2)
            emit_p2(1, 3)
            emit_vh(0)
            emit_vh(1)
            emit_p2(2, 0)
            emit_vh(2)
            emit_p2(2, 1)
            emit_vh(3)
            emit_p3(0, 0)
            emit_p2(2, 2)
            emit_p3(0, 1)
            emit_p2(2, 3)
            emit_zq(0, nc.vector)
            emit_stats(0, 1)
            emit_bsc(0, nc.gpsimd)
            emit_p2(3, 0)
            emit_p3(1, 0)
            emit_p2(3, 1)
            emit_p3(1, 1)
            emit_p5(0)
            emit_p2(3, 2)
            emit_zq(1, nc.vector)
            emit_stats(1, 1)
            emit_bsc(1, nc.gpsimd)
            emit_p6(0)
            emit_p2(3, 3)
            emit_p3(2, 0)
            emit_p5(1)
            emit_p3(2, 1)
            emit_p6(1)
            emit_zq(2, nc.vector)
            emit_stats(2, 1)
            emit_bsc(2, nc.gpsimd)
            emit_p3(3, 0)
            emit_p5(2)
            emit_p3(3, 1)
            emit_p6(2)
            emit_zq(3, nc.vector)
            emit_stats(3, 1)
            emit_bsc(3, nc.vector)
            emit_p5(3)
            emit_p6(3)

    return nc


def _build():
    import concourse.bass as bass  # noqa
    import concourse.tile as tile
    from concourse import bacc, mybir

    nc = bacc.Bacc("TRN2", target_bir_lowering=False, debug=False,
                   num_devices=NCORES)
    build_program(nc, tile, mybir)
    nc.compile()
    return nc


_cached_nc = None


def make_in_maps(q, k, v, Wq, bq, Wk, bk, Wv, bv, Wo, bo, alpha, beta):
    import ml_dtypes
    bft = ml_dtypes.bfloat16

    def prelay(x):
        xT = np.swapaxes(np.asarray(x, np.float32), 1, 2)  # [B, DM, S]
        return np.ascontiguousarray(
            xT.reshape(BS, 4, 128, S).transpose(0, 2, 1, 3)).astype(bft)

    def wlay(W):  # [DM, DK] -> [128, 4, DK]
        return np.ascontiguousarray(
            np.asarray(W, np.float32).reshape(4, 128, DK).transpose(1, 0, 2))

    qT, kT, vT = prelay(q), prelay(k), prelay(v)
    Wq, Wk, Wv, Wo = (np.asarray(x, np.float32) for x in (Wq, Wk, Wv, Wo))
    bq, bk, bv, bo = (np.asarray(x, np.float32) for x in (bq, bk, bv, bo))
    alpha, beta = np.asarray(alpha, np.float32), np.asarray(beta, np.float32)
    scale = np.float32(1.0 / np.sqrt(np.float32(DK)))
    idbf = np.eye(128, dtype=np.float32)
    in_maps = []
    for h in range(NCORES):
        sl = slice(h * DK, (h + 1) * DK)
        WkD = wlay(Wk[:, sl])
        WkD = np.concatenate([WkD, WkD], axis=2).reshape(128, 512)
        WqD = wlay(Wq[:, sl])
        WqD = np.concatenate([WqD, WqD], axis=2).reshape(128, 512)
        WvS = wlay(Wv[:, sl] * scale).reshape(128, 256)
        WoStack = np.concatenate([Wo[sl, :], 4.0 * Wo[sl, :]], axis=0)
        WoB = WoStack.reshape(128, 4, 128).reshape(128, 512)
        bvb = np.tile((bv[sl] * scale)[None, :], (128, 1))
        wblob = np.ascontiguousarray(np.concatenate(
            [WkD, WqD, WvS, WoB, bvb, idbf], axis=1)).astype(bft)
        bo_h = bo if h == 0 else np.zeros_like(bo)
        alx = np.zeros(128, np.float32)
        alx[0:DK] = alpha
        b4x = np.zeros(128, np.float32)
        b4x[0:DK] = 4.0 * beta
        fblob = np.ascontiguousarray(np.concatenate(
            [np.tile(bk[sl], 2)[:, None], np.tile(bq[sl], 2)[:, None],
             bo_h.reshape(4, 128).T, alx[:, None], b4x[:, None], idbf],
            axis=1)).astype(np.float32)
        in_maps.append({
            "wblob": wblob, "fblob": fblob,
            "kT": kT, "qT": qT, "vT": vT,
        })
    return in_maps


def assemble(results):
    out = np.zeros((BS, S, DM), np.float32)
    for r in results:
        out += np.asarray(r["outT"], np.float32).transpose(0, 3, 2, 1).reshape(
            BS, S, DM)
    return out


def kernel(**inputs) -> np.ndarray:
    global _cached_nc
    from concourse.bass_utils import run_bass_kernel_spmd

    if _cached_nc is None:
        _cached_nc = _build()
    in_maps = make_in_maps(**inputs)
    res = run_bass_kernel_spmd(_cached_nc, in_maps, list(range(NCORES)))
    return assemble(res.results)


# revision 25
# speedup vs baseline: 1.1057x; 1.0330x over previous
"""Trainium2 Bass kernel for nn_MultiHeadCrossAttention (BS=4, S=512, DM=512, H=8).

Sharding: one attention head per NeuronCore (8 heads / 8 cores). Each core
receives the full (transposed) q/k/v plus its head's weight slices, computes
its head end-to-end including the rank-64 slice of the output projection, and
the host sums the 8 partial outputs.

v2 layout/engine plan:
  - weights packed into one bf16 blob + one f32 blob, DMA'd first
  - k/q projections emit [128,*] dup outputs (lhsT = [W|W]) so P2 can run
    kb-pairs concurrently in row groups {0-63} / {64-127}
  - fenmu = sum_kb exp(.) via bf16 identity-matmul PSUM accumulation
  - softmax/LN stats via grouped bn_stats (even/odd combine) + 1-NR rsqrt
  - w0 applied pre-transpose as a per-partition scalar (no rank-1 matmuls)
  - P6 single K=128 matmul per chunk: lhsT = [Wo;4Wo], rhs = [heads;qh]
  - output DRAM layout [B,128,4,S] (4KB/partition descriptors)
"""

import numpy as np

BS, S, DM, H, DK = 4, 512, 512, 8, 64
NCORES = 8
WCOLS = 512 + 512 + 256 + 512 + 64 + 128  # Wk,Wq,Wv,WoB,bvb,id = 1984
FCOLS = 8 + 128  # biases + idf


def build_program(nc, tile, mybir):
    f32 = mybir.dt.float32
    bf16 = mybir.dt.bfloat16
    i32 = mybir.dt.int32
    AF = mybir.ActivationFunctionType
    OP = mybir.AluOpType

    wb_d = nc.dram_tensor("wblob", [128, WCOLS], bf16, kind="ExternalInput")
    fb_d = nc.dram_tensor("fblob", [128, FCOLS], f32, kind="ExternalInput")
    kT_d = nc.dram_tensor("kT", [BS, 128, 4, S], bf16, kind="ExternalInput")
    qT_d = nc.dram_tensor("qT", [BS, 128, 4, S], bf16, kind="ExternalInput")
    vT_d = nc.dram_tensor("vT", [BS, 128, 4, S], bf16, kind="ExternalInput")
    outT_d = nc.dram_tensor("outT", [BS, 128, 4, S], bf16, kind="ExternalOutput")

    with tile.TileContext(nc) as tc:
        with (
            tc.tile_pool(name="persist", bufs=1) as pp,
            tc.tile_pool(name="inp", bufs=3) as inp,
            tc.tile_pool(name="ex", bufs=2) as exp_,
            tc.tile_pool(name="wk", bufs=2) as wkp,
            tc.tile_pool(name="st", bufs=1) as stp,
            tc.tile_pool(name="ob", bufs=2) as obp,
            tc.tile_pool(name="psum", bufs=1, space="PSUM") as psp,
        ):
            # ---- persistent SBUF ----
            wb = pp.tile([128, WCOLS], bf16, tag="wb")
            fb = pp.tile([128, FCOLS], f32, tag="fb")
            Wk_s = wb[:, 0:512].rearrange("p (m c) -> p m c", m=4)
            Wq_s = wb[:, 512:1024].rearrange("p (m c) -> p m c", m=4)
            Wv_s = wb[:, 1024:1280].rearrange("p (m c) -> p m c", m=4)
            WoB_s = wb[:, 1280:1792].rearrange("p (m c) -> p m c", m=4)
            bvb_s = wb[:, 1792:1856]
            id_s = wb[:, 1856:1984]
            bk2 = fb[:, 0:1]
            bq2 = fb[:, 1:2]
            bo4 = fb[:, 2:6]
            alx = fb[:, 6:7]
            b4x = fb[:, 7:8]
            idf_s = fb[:, 8:136]

            khT2 = pp.tile([128, BS, S], bf16, tag="khT2")
            qhT2 = pp.tile([128, BS, S], bf16, tag="qhT2")
            vh_all = pp.tile([128, 4, BS, DK], bf16, tag="vh")  # [j,jc,c,d]
            heads = pp.tile([128, BS, S], bf16, tag="heads")  # 0:64 ln, 64: qh
            Z_all = pp.tile([128, BS, 16], f32, tag="Z")   # (b, ic*4+c)
            Q_all = pp.tile([128, BS, 16], f32, tag="Q")
            w1_all = pp.tile([128, BS, 16], f32, tag="w1")
            w0_all = pp.tile([128, 16], bf16, tag="w0")
            wup = pp.tile([1, 8], f32, tag="wup")

            def psum_pe():
                return psp.tile([128, 2, S], f32, tag="pe", bufs=2, name="ppe")

            def psum_fs(shape):
                return psp.tile(shape, f32, tag="fs", bufs=2, name="pfs")

            def psum_po(shape):
                return psp.tile(shape, f32, tag="po", bufs=2, name="ppo")

            # ---- prelude: ACT table preload + DMAs in use-order ----
            nc.vector.memset(wup[:], 1.0)
            nc.scalar.activation(wup[:], wup[:], AF.Exp)
            wmm = pp.tile([128, 128], bf16, tag="wmm")
            nc.gpsimd.memset(wmm[:], 0.0)
            nc.sync.dma_start(wb[:], wb_d[:])
            nc.sync.dma_start(fb[:], fb_d[:])
            ktiles = [inp.tile([128, 4, S], bf16, tag="kt", bufs=4,
                               name=f"kt{b}") for b in range(BS)]
            qtiles = [inp.tile([128, 4, S], bf16, tag="qt", bufs=4,
                               name=f"qt{b}") for b in range(BS)]
            vtiles = [inp.tile([128, 4, S], bf16, tag="vt", bufs=4,
                               name=f"vt{b}") for b in range(BS)]
            # trigger order = sync-engine issue order: wblob first, then k
            # (P2 needs all of kh), q0, then v and the remaining q's.
            nc.sync.dma_start(ktiles[0][:], kT_d[0])
            nc.sync.dma_start(ktiles[1][:], kT_d[1])
            nc.sync.dma_start(qtiles[0][:], qT_d[0])
            nc.sync.dma_start(ktiles[2][:], kT_d[2])
            nc.sync.dma_start(ktiles[3][:], kT_d[3])
            nc.sync.dma_start(qtiles[1][:], qT_d[1])
            nc.sync.dma_start(qtiles[2][:], qT_d[2])
            nc.sync.dma_start(qtiles[3][:], qT_d[3])
            nc.sync.dma_start(vtiles[0][:], vT_d[0])
            nc.sync.dma_start(vtiles[1][:], vT_d[1])
            nc.sync.dma_start(vtiles[2][:], vT_d[2])
            nc.sync.dma_start(vtiles[3][:], vT_d[3])

            # ---- phase emitters ----
            def emit_p1(W_s, bias_c, src, dst, b, alt=False):
                ps = (psp.tile([128, S], f32, tag="pe", bufs=2, name="ppe")
                      if alt else psum_po([128, S]))
                for mc in range(4):
                    nc.tensor.matmul(ps[:], W_s[:, mc, :], src[:, mc, :],
                                     start=(mc == 0), stop=(mc == 3))
                nc.vector.tensor_scalar(dst[:, b, :], ps[:], bias_c, None,
                                        op0=OP.add)

            def emit_hq(b):
                nc.vector.tensor_copy(heads[64:128, b, :], qhT2[64:128, b, :])

            def emit_vh(c):
                pv = psum_po([128, 4, DK])
                vt = vtiles[c]
                for jc in range(4):
                    for mc in range(4):
                        nc.tensor.matmul(
                            pv[:, jc, :], vt[:, mc, jc * 128:(jc + 1) * 128],
                            Wv_s[:, mc, :], start=(mc == 0), stop=(mc == 3))
                nc.vector.tensor_tensor(
                    vh_all[:, :, c, :], pv[:],
                    bvb_s[:].unsqueeze(1).broadcast_to((128, 4, DK)),
                    op=OP.add)

            rtiles = {}

            extiles = {}

            def emit_p2h(b, jc, half):
                jcs = slice(jc * 128, (jc + 1) * 128)
                if half == 0:
                    extiles[(b, jc)] = exp_.tile([128, 4, S], bf16, tag="ex",
                                                 bufs=4, name="exq")
                exq = extiles[(b, jc)]
                ph = psum_pe()
                kb0, kb1 = 2 * half, 2 * half + 1
                nc.tensor.matmul(ph[:, 0, :], khT2[0:64, kb0, jcs],
                                 qhT2[0:64, b, :], start=True, stop=True)
                nc.tensor.matmul(ph[:, 1, :], khT2[64:128, kb1, jcs],
                                 qhT2[64:128, b, :], start=True, stop=True)
                nc.scalar.activation(
                    exq[:, 2 * half:2 * half + 2, :], ph[:], AF.Exp)

            def emit_p2f(b, jc):
                exq = extiles[(b, jc)]
                fp = psum_fs([128, S])
                for kb in range(4):
                    nc.tensor.matmul(fp[:], id_s[:], exq[:, kb, :],
                                     start=(kb == 0), stop=(kb == 3))
                wrec = wkp.tile([128, S], f32, tag="wrec", bufs=3, name="wrec")
                nc.vector.reciprocal_approx_fast(wrec[:], fp[:])
                if jc == 0:
                    rtiles[b] = wkp.tile([128, 4, S], bf16, tag="rt",
                                         bufs=4, name=f"rt{b}")
                nc.gpsimd.tensor_tensor(rtiles[b][:, jc, :], exq[:, b, :],
                                        wrec[:], op=OP.mult)

            def emit_p2(b, jc):
                emit_p2h(b, jc, 0)
                emit_p2h(b, jc, 1)
                emit_p2f(b, jc)

            etiles = {}

            def emit_p3(b, p):
                rt = rtiles[b]
                if p == 0:
                    etiles[b] = wkp.tile([128, 4, BS, DK], bf16, tag="ea",
                                         bufs=4, name=f"ea{b}")
                ea = etiles[b]
                scp = psum_fs([128, 2, BS * DK])
                for u in range(2):
                    ic = 2 * p + u
                    ics = slice(ic * 128, (ic + 1) * 128)
                    for jc in range(4):
                        nc.tensor.matmul(
                            scp[:, u, :], rt[:, jc, ics],
                            vh_all[:, jc].rearrange("p c d -> p (c d)"),
                            start=(jc == 0), stop=(jc == 3))
                nc.scalar.activation(
                    ea[:, 2 * p:2 * p + 2].rearrange("p a c d -> p a (c d)"),
                    scp[:], AF.Exp)

            def emit_zq(b, eng):
                # e2 = e*e (gpsimd), then grouped reduces -> Z, Q (DVE-only)
                ea = etiles[b]
                e2 = wkp.tile([128, 16, DK], bf16, tag="e2", name="e2")
                eav = ea[:].rearrange("p a c d -> p (a c) d")
                eng.tensor_tensor(e2[:], eav, eav, op=OP.mult)
                nc.vector.tensor_reduce(Z_all[:, b, :], eav,
                                        axis=mybir.AxisListType.X, op=OP.add)
                nc.vector.tensor_reduce(Q_all[:, b, :], e2[:],
                                        axis=mybir.AxisListType.X, op=OP.add)

            SQ63 = float(np.sqrt(63.0))

            def emit_stats(b0, nb):
                # batches [b0, b0+nb): ln scale-invariance kills 1/Z:
                # w1 = sqrt(63)*rsqrt(Q - Z^2/64), w0 = -Z*w1/64
                bsl = slice(b0, b0 + nb)
                cnt = [128, 16 * nb]
                Zv = Z_all[:, bsl, :].rearrange("p b g -> p (b g)")
                Qv = Q_all[:, bsl, :].rearrange("p b g -> p (b g)")
                t_ = stp.tile(cnt, f32, tag=f"t{b0}", name="t_")
                nc.vector.tensor_tensor(t_[:], Zv, Zv, op=OP.mult)
                s_ = stp.tile(cnt, f32, tag=f"s{b0}", name="s_")
                nc.vector.scalar_tensor_tensor(s_[:], t_[:], -1.0 / DK, Qv,
                                               op0=OP.mult, op1=OP.add)
                # rsqrt seed + 1 NR iter (w1 fused into the final mult)
                r_ = stp.tile(cnt, f32, tag=f"r{b0}", name="r_")
                nc.vector.tensor_scalar(r_[:].bitcast(i32), s_[:].bitcast(i32),
                                        1, None, op0=OP.logical_shift_right)
                nc.vector.tensor_scalar(r_[:].bitcast(i32), r_[:].bitcast(i32),
                                        -1, 0x5F3759DF, op0=OP.mult, op1=OP.add)
                nt = stp.tile(cnt, f32, tag=f"n{b0}", name="nt")
                nc.vector.tensor_tensor(nt[:], s_[:], r_[:], op=OP.mult)
                nc.vector.tensor_tensor(nt[:], nt[:], r_[:], op=OP.mult)
                nc.vector.tensor_scalar(nt[:], nt[:], -0.5, 1.5,
                                        op0=OP.mult, op1=OP.add)
                w1v = w1_all[:, bsl, :].rearrange("p b g -> p (b g)")
                nc.vector.scalar_tensor_tensor(w1v, r_[:], SQ63, nt[:],
                                               op0=OP.mult, op1=OP.mult)
                zg = stp.tile(cnt, f32, tag=f"zg{b0}", name="zg")
                nc.vector.scalar_tensor_tensor(zg[:], Zv, -1.0 / DK, w1v,
                                               op0=OP.mult, op1=OP.mult)
                w0r = stp.tile([128, 4 * nb], f32, tag=f"w{b0}", name="w0r")
                nc.vector.tensor_reduce(
                    w0r[:], zg[:].rearrange("p (g c) -> p g c", c=4),
                    axis=mybir.AxisListType.X, op=OP.add)
                nc.vector.tensor_copy(w0_all[:, 4 * b0:4 * (b0 + nb)], w0r[:])

            bsctiles = {}

            def emit_bsc(b, eng):
                ea = etiles[b]
                bsc = obp.tile([128, 16, DK], bf16, tag="bsc", bufs=4,
                               name=f"bsc{b}")
                bsctiles[b] = bsc
                w1b = (w1_all[:, b, :].unsqueeze(-1)
                       .broadcast_to((128, 16, DK)))
                eng.tensor_tensor(
                    bsc[:], ea[:].rearrange("p a c d -> p (a c) d"), w1b,
                    op=OP.mult)

            def emit_p5(b):
                bsc = bsctiles[b]
                bp = psum_fs([128, 4, DK])
                for ic in range(4):
                    for c in range(4):
                        nc.tensor.matmul(bp[:, ic, :], id_s[:],
                                         bsc[:, ic * 4 + c, :],
                                         start=(c == 0), stop=False)
                    # += w0 broadcast along d via a 0-stride rhs
                    nc.tensor.matmul(
                        bp[:, ic, :], id_s[:],
                        w0_all[:, b * 4 + ic:b * 4 + ic + 1]
                        .broadcast_to((128, DK)),
                        start=False, stop=True)
                balls = obp.tile([128, 4, DK], f32, tag="balls", name="balls")
                nc.vector.tensor_copy(balls[:], bp[:])
                pt = psum_po([64, S])
                for ic in range(4):
                    nc.tensor.matmul(pt[0:64, ic * 128:(ic + 1) * 128],
                                     balls[:, ic, :], idf_s,
                                     is_transpose=True, start=True, stop=True)
                if b % 2 == 0:
                    nc.scalar.activation(heads[0:64, b, :], pt[0:64, :],
                                         AF.Identity, bias=b4x[0:64, :],
                                         scale=alx[0:64, :])
                else:
                    nc.vector.tensor_scalar(heads[0:64, b, :], pt[0:64, :],
                                            alx[0:64, :], b4x[0:64, :],
                                            op0=OP.mult, op1=OP.add)

            def emit_p6(b):
                osb = obp.tile([128, 4, S], bf16, tag="osb", name="osb")
                for nch in range(4):
                    pp6 = psum_po([128, S])
                    nc.tensor.matmul(pp6[:], WoB_s[:, nch, :], heads[:, b, :],
                                     start=True, stop=True)
                    if nch % 2 == 0:
                        nc.scalar.activation(osb[:, nch, :], pp6[:],
                                             AF.Identity,
                                             bias=bo4[:, nch:nch + 1])
                    else:
                        nc.vector.tensor_scalar(osb[:, nch, :], pp6[:],
                                                bo4[:, nch:nch + 1], None,
                                                op0=OP.add)
                nc.sync.dma_start(outT_d[b], osb[:])

            # ---- emission schedule (engine queues are FIFO) ----
            # PE warmup (HAM) on junk data while DMA streams
            for _ in range(14):
                pw_ = psp.tile([128, 128], f32, tag="po", bufs=2, name="pwm")
                nc.tensor.matmul(pw_[:], wmm[:], wmm[:], start=True, stop=True)
            emit_p1(Wk_s, bk2, ktiles[0], khT2, 0)
            emit_p1(Wk_s, bk2, ktiles[1], khT2, 1, alt=True)
            emit_p1(Wq_s, bq2, qtiles[0], qhT2, 0)
            emit_hq(0)
            emit_p2h(0, 0, 0)
            emit_p2h(0, 1, 0)
            emit_p1(Wk_s, bk2, ktiles[2], khT2, 2, alt=True)
            emit_p2h(0, 2, 0)
            emit_p1(Wk_s, bk2, ktiles[3], khT2, 3)
            emit_p2h(0, 3, 0)
            emit_p2h(0, 0, 1)
            emit_p2f(0, 0)
            emit_p1(Wq_s, bq2, qtiles[1], qhT2, 1)
            emit_hq(1)
            emit_p2h(0, 1, 1)
            emit_p2f(0, 1)
            emit_p2h(0, 2, 1)
            emit_p2f(0, 2)
            emit_p1(Wq_s, bq2, qtiles[2], qhT2, 2)
            emit_hq(2)
            emit_p2h(0, 3, 1)
            emit_p2f(0, 3)
            emit_p2(1, 0)
            emit_p1(Wq_s, bq2, qtiles[3], qhT2, 3)
            emit_hq(3)
            emit_p2(1, 1)
            emit_p2(1, 2)
            emit_p2(1, 3)
            emit_vh(0)
            emit_vh(1)
            emit_p2(2, 0)
            emit_vh(2)
            emit_p2(2, 1)
            emit_vh(3)
            emit_p3(0, 0)
            emit_p2(2, 2)
            emit_p3(0, 1)
            emit_p2(2, 3)
            emit_p3(1, 0)
            emit_p2(3, 0)
            emit_p3(1, 1)
            emit_p2(3, 1)
            emit_p2(3, 2)
            emit_p2(3, 3)
            emit_zq(0, nc.vector)
            emit_stats(0, 1)
            emit_bsc(0, nc.gpsimd)
            emit_p3(2, 0)
            emit_p5(0)
            emit_p3(2, 1)
            emit_p6(0)
            emit_zq(1, nc.vector)
            emit_stats(1, 1)
            emit_bsc(1, nc.gpsimd)
            emit_p3(3, 0)
            emit_p5(1)
            emit_p3(3, 1)
            emit_p6(1)
            emit_zq(2, nc.vector)
            emit_stats(2, 1)
            emit_bsc(2, nc.gpsimd)
            emit_p5(2)
            emit_p6(2)
            emit_zq(3, nc.vector)
            emit_stats(3, 1)
            emit_bsc(3, nc.gpsimd)
            emit_p5(3)
            emit_p6(3)

    return nc


def _build():
    import concourse.bass as bass  # noqa
    import concourse.tile as tile
    from concourse import bacc, mybir

    nc = bacc.Bacc("TRN2", target_bir_lowering=False, debug=False,
                   num_devices=NCORES)
    build_program(nc, tile, mybir)
    nc.compile()
    return nc


_cached_nc = None


def make_in_maps(q, k, v, Wq, bq, Wk, bk, Wv, bv, Wo, bo, alpha, beta):
    import ml_dtypes
    bft = ml_dtypes.bfloat16

    def prelay(x):
        xT = np.swapaxes(np.asarray(x, np.float32), 1, 2)  # [B, DM, S]
        return np.ascontiguousarray(
            xT.reshape(BS, 4, 128, S).transpose(0, 2, 1, 3)).astype(bft)

    def wlay(W):  # [DM, DK] -> [128, 4, DK]
        return np.ascontiguousarray(
            np.asarray(W, np.float32).reshape(4, 128, DK).transpose(1, 0, 2))

    qT, kT, vT = prelay(q), prelay(k), prelay(v)
    Wq, Wk, Wv, Wo = (np.asarray(x, np.float32) for x in (Wq, Wk, Wv, Wo))
    bq, bk, bv, bo = (np.asarray(x, np.float32) for x in (bq, bk, bv, bo))
    alpha, beta = np.asarray(alpha, np.float32), np.asarray(beta, np.float32)
    scale = np.float32(1.0 / np.sqrt(np.float32(DK)))
    idbf = np.eye(128, dtype=np.float32)
    in_maps = []
    for h in range(NCORES):
        sl = slice(h * DK, (h + 1) * DK)
        WkD = wlay(Wk[:, sl])
        WkD = np.concatenate([WkD, WkD], axis=2).reshape(128, 512)
        WqD = wlay(Wq[:, sl])
        WqD = np.concatenate([WqD, WqD], axis=2).reshape(128, 512)
        WvS = wlay(Wv[:, sl] * scale).reshape(128, 256)
        WoStack = np.concatenate([Wo[sl, :], 4.0 * Wo[sl, :]], axis=0)
        WoB = WoStack.reshape(128, 4, 128).reshape(128, 512)
        bvb = np.tile((bv[sl] * scale)[None, :], (128, 1))
        wblob = np.ascontiguousarray(np.concatenate(
            [WkD, WqD, WvS, WoB, bvb, idbf], axis=1)).astype(bft)
        bo_h = bo if h == 0 else np.zeros_like(bo)
        alx = np.zeros(128, np.float32)
        alx[0:DK] = alpha
        b4x = np.zeros(128, np.float32)
        b4x[0:DK] = 4.0 * beta
        fblob = np.ascontiguousarray(np.concatenate(
            [np.tile(bk[sl], 2)[:, None], np.tile(bq[sl], 2)[:, None],
             bo_h.reshape(4, 128).T, alx[:, None], b4x[:, None], idbf],
            axis=1)).astype(np.float32)
        in_maps.append({
            "wblob": wblob, "fblob": fblob,
            "kT": kT, "qT": qT, "vT": vT,
        })
    return in_maps


def assemble(results):
    out = np.zeros((BS, S, DM), np.float32)
    for r in results:
        out += np.asarray(r["outT"], np.float32).transpose(0, 3, 2, 1).reshape(
            BS, S, DM)
    return out


def kernel(**inputs) -> np.ndarray:
    global _cached_nc
    from concourse.bass_utils import run_bass_kernel_spmd

    if _cached_nc is None:
        _cached_nc = _build()
    in_maps = make_in_maps(**inputs)
    res = run_bass_kernel_spmd(_cached_nc, in_maps, list(range(NCORES)))
    return assemble(res.results)


# revision 26
# speedup vs baseline: 1.1385x; 1.0297x over previous
"""Trainium2 Bass kernel for nn_MultiHeadCrossAttention (BS=4, S=512, DM=512, H=8).

Sharding: one attention head per NeuronCore (8 heads / 8 cores). Each core
receives the full (transposed) q/k/v plus its head's weight slices, computes
its head end-to-end including the rank-64 slice of the output projection, and
the host sums the 8 partial outputs.

v2 layout/engine plan:
  - weights packed into one bf16 blob + one f32 blob, DMA'd first
  - k/q projections emit [128,*] dup outputs (lhsT = [W|W]) so P2 can run
    kb-pairs concurrently in row groups {0-63} / {64-127}
  - fenmu = sum_kb exp(.) via bf16 identity-matmul PSUM accumulation
  - softmax/LN stats via grouped bn_stats (even/odd combine) + 1-NR rsqrt
  - w0 applied pre-transpose as a per-partition scalar (no rank-1 matmuls)
  - P6 single K=128 matmul per chunk: lhsT = [Wo;4Wo], rhs = [heads;qh]
  - output DRAM layout [B,128,4,S] (4KB/partition descriptors)
"""

import numpy as np

BS, S, DM, H, DK = 4, 512, 512, 8, 64
NCORES = 8
WCOLS = 512 + 512 + 256 + 512 + 64 + 128  # Wk,Wq,Wv,WoB,bvb,id = 1984
FCOLS = 8 + 128  # biases + idf


def build_program(nc, tile, mybir):
    f32 = mybir.dt.float32
    bf16 = mybir.dt.bfloat16
    i32 = mybir.dt.int32
    AF = mybir.ActivationFunctionType
    OP = mybir.AluOpType

    wb_d = nc.dram_tensor("wblob", [128, WCOLS], bf16, kind="ExternalInput")
    fb_d = nc.dram_tensor("fblob", [128, FCOLS], f32, kind="ExternalInput")
    kT_d = nc.dram_tensor("kT", [BS, 128, 4, S], bf16, kind="ExternalInput")
    qT_d = nc.dram_tensor("qT", [BS, 128, 4, S], bf16, kind="ExternalInput")
    vT_d = nc.dram_tensor("vT", [BS, 128, 4, S], bf16, kind="ExternalInput")
    outT_d = nc.dram_tensor("outT", [BS, 128, 4, S], bf16, kind="ExternalOutput")

    with tile.TileContext(nc) as tc:
        with (
            tc.tile_pool(name="persist", bufs=1) as pp,
            tc.tile_pool(name="inp", bufs=3) as inp,
            tc.tile_pool(name="ex", bufs=2) as exp_,
            tc.tile_pool(name="wk", bufs=2) as wkp,
            tc.tile_pool(name="st", bufs=1) as stp,
            tc.tile_pool(name="ob", bufs=2) as obp,
            tc.tile_pool(name="psum", bufs=1, space="PSUM") as psp,
        ):
            # ---- persistent SBUF ----
            wb = pp.tile([128, WCOLS], bf16, tag="wb")
            fb = pp.tile([128, FCOLS], f32, tag="fb")
            Wk_s = wb[:, 0:512].rearrange("p (m c) -> p m c", m=4)
            Wq_s = wb[:, 512:1024].rearrange("p (m c) -> p m c", m=4)
            Wv_s = wb[:, 1024:1280].rearrange("p (m c) -> p m c", m=4)
            WoB_s = wb[:, 1280:1792].rearrange("p (m c) -> p m c", m=4)
            bvb_s = wb[:, 1792:1856]
            id_s = wb[:, 1856:1984]
            bk2 = fb[:, 0:1]
            bq2 = fb[:, 1:2]
            bo4 = fb[:, 2:6]
            alx = fb[:, 6:7]
            b4x = fb[:, 7:8]
            idf_s = fb[:, 8:136]

            khT2 = pp.tile([128, BS, S], bf16, tag="khT2")
            qhT2 = pp.tile([128, BS, S], bf16, tag="qhT2")
            vh_all = pp.tile([128, 4, BS, DK], bf16, tag="vh")  # [j,jc,c,d]
            heads = pp.tile([128, BS, S], bf16, tag="heads")  # 0:64 ln, 64: qh
            Z_all = pp.tile([128, BS, 16], f32, tag="Z")   # (b, ic*4+c)
            Q_all = pp.tile([128, BS, 16], f32, tag="Q")
            w1_all = pp.tile([128, BS, 16], f32, tag="w1")
            w0_all = pp.tile([128, 16], bf16, tag="w0")
            wup = pp.tile([1, 8], f32, tag="wup")

            def psum_pe():
                return psp.tile([128, 2, S], f32, tag="pe", bufs=2, name="ppe")

            def psum_fs(shape):
                return psp.tile(shape, f32, tag="fs", bufs=2, name="pfs")

            def psum_po(shape):
                return psp.tile(shape, f32, tag="po", bufs=2, name="ppo")

            # ---- prelude: ACT table preload + DMAs in use-order ----
            nc.vector.memset(wup[:], 1.0)
            nc.scalar.activation(wup[:], wup[:], AF.Exp)
            wmm = pp.tile([128, 128], bf16, tag="wmm")
            nc.gpsimd.memset(wmm[:], 0.0)
            nc.sync.dma_start(wb[:], wb_d[:])
            nc.sync.dma_start(fb[:], fb_d[:])
            ktiles = [inp.tile([128, 4, S], bf16, tag="kt", bufs=4,
                               name=f"kt{b}") for b in range(BS)]
            qtiles = [inp.tile([128, 4, S], bf16, tag="qt", bufs=4,
                               name=f"qt{b}") for b in range(BS)]
            vtiles = [inp.tile([128, 4, S], bf16, tag="vt", bufs=4,
                               name=f"vt{b}") for b in range(BS)]
            # trigger order = sync-engine issue order: wblob first, then k
            # (P2 needs all of kh), q0, then v and the remaining q's.
            nc.sync.dma_start(ktiles[0][:], kT_d[0])
            nc.sync.dma_start(ktiles[1][:], kT_d[1])
            nc.sync.dma_start(qtiles[0][:], qT_d[0])
            nc.sync.dma_start(ktiles[2][:], kT_d[2])
            nc.sync.dma_start(ktiles[3][:], kT_d[3])
            nc.sync.dma_start(qtiles[1][:], qT_d[1])
            nc.sync.dma_start(qtiles[2][:], qT_d[2])
            nc.sync.dma_start(qtiles[3][:], qT_d[3])
            nc.sync.dma_start(vtiles[0][:], vT_d[0])
            nc.sync.dma_start(vtiles[1][:], vT_d[1])
            nc.sync.dma_start(vtiles[2][:], vT_d[2])
            nc.sync.dma_start(vtiles[3][:], vT_d[3])

            # ---- phase emitters ----
            def emit_p1(W_s, bias_c, src, dst, b, alt=False):
                ps = (psp.tile([128, S], f32, tag="pe", bufs=2, name="ppe")
                      if alt else psum_po([128, S]))
                for mc in range(4):
                    nc.tensor.matmul(ps[:], W_s[:, mc, :], src[:, mc, :],
                                     start=(mc == 0), stop=(mc == 3))
                nc.vector.tensor_scalar(dst[:, b, :], ps[:], bias_c, None,
                                        op0=OP.add)

            def emit_hq(b):
                nc.vector.tensor_copy(heads[64:128, b, :], qhT2[64:128, b, :])

            def emit_vh(c):
                pv = psum_po([128, 4, DK])
                vt = vtiles[c]
                for jc in range(4):
                    for mc in range(4):
                        nc.tensor.matmul(
                            pv[:, jc, :], vt[:, mc, jc * 128:(jc + 1) * 128],
                            Wv_s[:, mc, :], start=(mc == 0), stop=(mc == 3))
                nc.vector.tensor_tensor(
                    vh_all[:, :, c, :], pv[:],
                    bvb_s[:].unsqueeze(1).broadcast_to((128, 4, DK)),
                    op=OP.add)

            rtiles = {}

            extiles = {}

            def emit_p2h(b, jc, half):
                jcs = slice(jc * 128, (jc + 1) * 128)
                if half == 0:
                    extiles[(b, jc)] = exp_.tile([128, 4, S], bf16, tag="ex",
                                                 bufs=4, name="exq")
                exq = extiles[(b, jc)]
                ph = psum_pe()
                kb0, kb1 = 2 * half, 2 * half + 1
                nc.tensor.matmul(ph[:, 0, :], khT2[0:64, kb0, jcs],
                                 qhT2[0:64, b, :], start=True, stop=True)
                nc.tensor.matmul(ph[:, 1, :], khT2[64:128, kb1, jcs],
                                 qhT2[64:128, b, :], start=True, stop=True)
                nc.scalar.activation(
                    exq[:, 2 * half:2 * half + 2, :], ph[:], AF.Exp)

            def emit_p2f(b, jc):
                exq = extiles[(b, jc)]
                fp = psum_fs([128, S])
                for kb in range(4):
                    nc.tensor.matmul(fp[:], id_s[:], exq[:, kb, :],
                                     start=(kb == 0), stop=(kb == 3))
                wrec = wkp.tile([128, S], f32, tag="wrec", bufs=3, name="wrec")
                nc.vector.reciprocal_approx_fast(wrec[:], fp[:])
                if jc == 0:
                    rtiles[b] = wkp.tile([128, 4, S], bf16, tag="rt",
                                         bufs=4, name=f"rt{b}")
                nc.gpsimd.tensor_tensor(rtiles[b][:, jc, :], exq[:, b, :],
                                        wrec[:], op=OP.mult)

            def emit_p2(b, jc):
                emit_p2h(b, jc, 0)
                emit_p2h(b, jc, 1)
                emit_p2f(b, jc)

            etiles = {}

            def emit_p3(b, p):
                rt = rtiles[b]
                if p == 0:
                    etiles[b] = wkp.tile([128, 4, BS, DK], bf16, tag="ea",
                                         bufs=4, name=f"ea{b}")
                ea = etiles[b]
                scp = psum_fs([128, 2, BS * DK])
                for u in range(2):
                    ic = 2 * p + u
                    ics = slice(ic * 128, (ic + 1) * 128)
                    for jc in range(4):
                        nc.tensor.matmul(
                            scp[:, u, :], rt[:, jc, ics],
                            vh_all[:, jc].rearrange("p c d -> p (c d)"),
                            start=(jc == 0), stop=(jc == 3))
                nc.scalar.activation(
                    ea[:, 2 * p:2 * p + 2].rearrange("p a c d -> p a (c d)"),
                    scp[:], AF.Exp)

            def emit_zq(b, eng):
                # e2 = e*e (gpsimd), then grouped reduces -> Z, Q (DVE-only)
                ea = etiles[b]
                e2 = wkp.tile([128, 16, DK], bf16, tag="e2", name="e2")
                eav = ea[:].rearrange("p a c d -> p (a c) d")
                eng.tensor_tensor(e2[:], eav, eav, op=OP.mult)
                nc.vector.tensor_reduce(Z_all[:, b, :], eav,
                                        axis=mybir.AxisListType.X, op=OP.add)
                nc.vector.tensor_reduce(Q_all[:, b, :], e2[:],
                                        axis=mybir.AxisListType.X, op=OP.add)

            SQ63 = float(np.sqrt(63.0))

            def emit_stats(b0, nb):
                # batches [b0, b0+nb): ln scale-invariance kills 1/Z:
                # w1 = sqrt(63)*rsqrt(Q - Z^2/64), w0 = -Z*w1/64
                bsl = slice(b0, b0 + nb)
                cnt = [128, 16 * nb]
                Zv = Z_all[:, bsl, :].rearrange("p b g -> p (b g)")
                Qv = Q_all[:, bsl, :].rearrange("p b g -> p (b g)")
                t_ = stp.tile(cnt, f32, tag=f"t{b0}", name="t_")
                nc.vector.tensor_tensor(t_[:], Zv, Zv, op=OP.mult)
                s_ = stp.tile(cnt, f32, tag=f"s{b0}", name="s_")
                nc.vector.scalar_tensor_tensor(s_[:], t_[:], -1.0 / DK, Qv,
                                               op0=OP.mult, op1=OP.add)
                # rsqrt seed + 1 NR iter (w1 fused into the final mult)
                r_ = stp.tile(cnt, f32, tag=f"r{b0}", name="r_")
                nc.vector.tensor_scalar(r_[:].bitcast(i32), s_[:].bitcast(i32),
                                        1, None, op0=OP.logical_shift_right)
                nc.vector.tensor_scalar(r_[:].bitcast(i32), r_[:].bitcast(i32),
                                        -1, 0x5F3759DF, op0=OP.mult, op1=OP.add)
                nt = stp.tile(cnt, f32, tag=f"n{b0}", name="nt")
                nc.vector.tensor_tensor(nt[:], s_[:], r_[:], op=OP.mult)
                nc.vector.tensor_tensor(nt[:], nt[:], r_[:], op=OP.mult)
                nc.vector.tensor_scalar(nt[:], nt[:], -0.5, 1.5,
                                        op0=OP.mult, op1=OP.add)
                w1v = w1_all[:, bsl, :].rearrange("p b g -> p (b g)")
                nc.vector.scalar_tensor_tensor(w1v, r_[:], SQ63, nt[:],
                                               op0=OP.mult, op1=OP.mult)
                zg = stp.tile(cnt, f32, tag=f"zg{b0}", name="zg")
                nc.vector.scalar_tensor_tensor(zg[:], Zv, -1.0 / DK, w1v,
                                               op0=OP.mult, op1=OP.mult)
                w0r = stp.tile([128, 4 * nb], f32, tag=f"w{b0}", name="w0r")
                nc.vector.tensor_reduce(
                    w0r[:], zg[:].rearrange("p (g c) -> p g c", c=4),
                    axis=mybir.AxisListType.X, op=OP.add)
                nc.vector.tensor_copy(w0_all[:, 4 * b0:4 * (b0 + nb)], w0r[:])

            bsctiles = {}

            def emit_bsc(b, eng):
                ea = etiles[b]
                bsc = obp.tile([128, 16, DK], bf16, tag="bsc", bufs=4,
                               name=f"bsc{b}")
                bsctiles[b] = bsc
                w1b = (w1_all[:, b, :].unsqueeze(-1)
                       .broadcast_to((128, 16, DK)))
                eng.tensor_tensor(
                    bsc[:], ea[:].rearrange("p a c d -> p (a c) d"), w1b,
                    op=OP.mult)

            def emit_p5(b, heads_act=True):
                bsc = bsctiles[b]
                bp = psum_fs([128, 4, DK])
                for ic in range(4):
                    for c in range(4):
                        nc.tensor.matmul(bp[:, ic, :], id_s[:],
                                         bsc[:, ic * 4 + c, :],
                                         start=(c == 0), stop=False)
                    # += w0 broadcast along d via a 0-stride rhs
                    nc.tensor.matmul(
                        bp[:, ic, :], id_s[:],
                        w0_all[:, b * 4 + ic:b * 4 + ic + 1]
                        .broadcast_to((128, DK)),
                        start=False, stop=True)
                balls = obp.tile([128, 4, DK], f32, tag="balls", name="balls")
                nc.vector.tensor_copy(balls[:], bp[:])
                pt = psum_po([64, S])
                for ic in range(4):
                    nc.tensor.matmul(pt[0:64, ic * 128:(ic + 1) * 128],
                                     balls[:, ic, :], idf_s,
                                     is_transpose=True, start=True, stop=True)
                if heads_act:
                    nc.scalar.activation(heads[0:64, b, :], pt[0:64, :],
                                         AF.Identity, bias=b4x[0:64, :],
                                         scale=alx[0:64, :])
                else:
                    nc.vector.tensor_scalar(heads[0:64, b, :], pt[0:64, :],
                                            alx[0:64, :], b4x[0:64, :],
                                            op0=OP.mult, op1=OP.add)

            def emit_p6(b, act_all=False):
                osb = obp.tile([128, 4, S], bf16, tag="osb", name="osb")
                for nch in range(4):
                    pp6 = psum_po([128, S])
                    nc.tensor.matmul(pp6[:], WoB_s[:, nch, :], heads[:, b, :],
                                     start=True, stop=True)
                    if act_all or nch % 2 == 0:
                        nc.scalar.activation(osb[:, nch, :], pp6[:],
                                             AF.Identity,
                                             bias=bo4[:, nch:nch + 1])
                    else:
                        nc.vector.tensor_scalar(osb[:, nch, :], pp6[:],
                                                bo4[:, nch:nch + 1], None,
                                                op0=OP.add)
                nc.sync.dma_start(outT_d[b], osb[:])

            # ---- emission schedule (engine queues are FIFO) ----
            # PE warmup (HAM) on junk data while DMA streams
            for _ in range(14):
                pw_ = psp.tile([128, 128], f32, tag="po", bufs=2, name="pwm")
                nc.tensor.matmul(pw_[:], wmm[:], wmm[:], start=True, stop=True)
            emit_p1(Wk_s, bk2, ktiles[0], khT2, 0)
            emit_p1(Wk_s, bk2, ktiles[1], khT2, 1, alt=True)
            emit_p1(Wq_s, bq2, qtiles[0], qhT2, 0)
            emit_hq(0)
            emit_p2h(0, 0, 0)
            emit_p2h(0, 1, 0)
            emit_p1(Wk_s, bk2, ktiles[2], khT2, 2, alt=True)
            emit_p2h(0, 2, 0)
            emit_p1(Wk_s, bk2, ktiles[3], khT2, 3)
            emit_p2h(0, 3, 0)
            emit_p2h(0, 0, 1)
            emit_p2f(0, 0)
            emit_p1(Wq_s, bq2, qtiles[1], qhT2, 1)
            emit_hq(1)
            emit_p2h(0, 1, 1)
            emit_p2f(0, 1)
            emit_p2h(0, 2, 1)
            emit_p2f(0, 2)
            emit_p1(Wq_s, bq2, qtiles[2], qhT2, 2)
            emit_hq(2)
            emit_p2h(0, 3, 1)
            emit_p2f(0, 3)
            emit_p2(1, 0)
            emit_p1(Wq_s, bq2, qtiles[3], qhT2, 3)
            emit_hq(3)
            emit_p2(1, 1)
            emit_p2(1, 2)
            emit_p2(1, 3)
            emit_vh(0)
            emit_vh(1)
            emit_p2(2, 0)
            emit_vh(2)
            emit_p2(2, 1)
            emit_vh(3)
            emit_p3(0, 0)
            emit_p2(2, 2)
            emit_p3(0, 1)
            emit_p2(2, 3)
            emit_p3(1, 0)
            emit_p2(3, 0)
            emit_p3(1, 1)
            emit_p2(3, 1)
            emit_p2(3, 2)
            emit_p2(3, 3)
            emit_zq(0, nc.vector)
            emit_stats(0, 1)
            emit_bsc(0, nc.gpsimd)
            emit_p3(2, 0)
            emit_p5(0)
            emit_p3(2, 1)
            emit_p6(0)
            emit_zq(1, nc.vector)
            emit_stats(1, 1)
            emit_bsc(1, nc.gpsimd)
            emit_p3(3, 0)
            emit_p5(1, heads_act=False)
            emit_p3(3, 1)
            emit_p6(1)
            emit_zq(2, nc.vector)
            emit_stats(2, 1)
            emit_bsc(2, nc.gpsimd)
            emit_p5(2)
            emit_zq(3, nc.vector)
            emit_stats(3, 1)
            emit_p6(2, act_all=True)
            emit_bsc(3, nc.gpsimd)
            emit_p5(3)
            emit_p6(3, act_all=True)

    return nc


def _build():
    import concourse.bass as bass  # noqa
    import concourse.tile as tile
    from concourse import bacc, mybir

    nc = bacc.Bacc("TRN2", target_bir_lowering=False, debug=False,
                   num_devices=NCORES)
    build_program(nc, tile, mybir)
    nc.compile()
    return nc


_cached_nc = None


def make_in_maps(q, k, v, Wq, bq, Wk, bk, Wv, bv, Wo, bo, alpha, beta):
    import ml_dtypes
    bft = ml_dtypes.bfloat16

    def prelay(x):
        xT = np.swapaxes(np.asarray(x, np.float32), 1, 2)  # [B, DM, S]
        return np.ascontiguousarray(
            xT.reshape(BS, 4, 128, S).transpose(0, 2, 1, 3)).astype(bft)

    def wlay(W):  # [DM, DK] -> [128, 4, DK]
        return np.ascontiguousarray(
            np.asarray(W, np.float32).reshape(4, 128, DK).transpose(1, 0, 2))

    qT, kT, vT = prelay(q), prelay(k), prelay(v)
    Wq, Wk, Wv, Wo = (np.asarray(x, np.float32) for x in (Wq, Wk, Wv, Wo))
    bq, bk, bv, bo = (np.asarray(x, np.float32) for x in (bq, bk, bv, bo))
    alpha, beta = np.asarray(alpha, np.float32), np.asarray(beta, np.float32)
    scale = np.float32(1.0 / np.sqrt(np.float32(DK)))
    idbf = np.eye(128, dtype=np.float32)
    in_maps = []
    for h in range(NCORES):
        sl = slice(h * DK, (h + 1) * DK)
        WkD = wlay(Wk[:, sl])
        WkD = np.concatenate([WkD, WkD], axis=2).reshape(128, 512)
        WqD = wlay(Wq[:, sl])
        WqD = np.concatenate([WqD, WqD], axis=2).reshape(128, 512)
        WvS = wlay(Wv[:, sl] * scale).reshape(128, 256)
        WoStack = np.concatenate([Wo[sl, :], 4.0 * Wo[sl, :]], axis=0)
        WoB = WoStack.reshape(128, 4, 128).reshape(128, 512)
        bvb = np.tile((bv[sl] * scale)[None, :], (128, 1))
        wblob = np.ascontiguousarray(np.concatenate(
            [WkD, WqD, WvS, WoB, bvb, idbf], axis=1)).astype(bft)
        bo_h = bo if h == 0 else np.zeros_like(bo)
        alx = np.zeros(128, np.float32)
        alx[0:DK] = alpha
        b4x = np.zeros(128, np.float32)
        b4x[0:DK] = 4.0 * beta
        fblob = np.ascontiguousarray(np.concatenate(
            [np.tile(bk[sl], 2)[:, None], np.tile(bq[sl], 2)[:, None],
             bo_h.reshape(4, 128).T, alx[:, None], b4x[:, None], idbf],
            axis=1)).astype(np.float32)
        in_maps.append({
            "wblob": wblob, "fblob": fblob,
            "kT": kT, "qT": qT, "vT": vT,
        })
    return in_maps


def assemble(results):
    out = np.zeros((BS, S, DM), np.float32)
    for r in results:
        out += np.asarray(r["outT"], np.float32).transpose(0, 3, 2, 1).reshape(
            BS, S, DM)
    return out


def kernel(**inputs) -> np.ndarray:
    global _cached_nc
    from concourse.bass_utils import run_bass_kernel_spmd

    if _cached_nc is None:
        _cached_nc = _build()
    in_maps = make_in_maps(**inputs)
    res = run_bass_kernel_spmd(_cached_nc, in_maps, list(range(NCORES)))
    return assemble(res.results)


# revision 29
# speedup vs baseline: 1.2113x; 1.0639x over previous
"""Trainium2 Bass kernel for nn_MultiHeadCrossAttention (BS=4, S=512, DM=512, H=8).

Sharding: one attention head per NeuronCore (8 heads / 8 cores). Each core
receives the full (transposed) q/k/v plus its head's weight slices, computes
its head end-to-end including the rank-64 slice of the output projection, and
the host sums the 8 partial outputs.

v2 layout/engine plan:
  - weights packed into one bf16 blob + one f32 blob, DMA'd first
  - k/q projections emit [128,*] dup outputs (lhsT = [W|W]) so P2 can run
    kb-pairs concurrently in row groups {0-63} / {64-127}
  - fenmu = sum_kb exp(.) via bf16 identity-matmul PSUM accumulation
  - softmax/LN stats via grouped bn_stats (even/odd combine) + 1-NR rsqrt
  - w0 applied pre-transpose as a per-partition scalar (no rank-1 matmuls)
  - P6 single K=128 matmul per chunk: lhsT = [Wo;4Wo], rhs = [heads;qh]
  - output DRAM layout [B,128,4,S] (4KB/partition descriptors)
"""

import numpy as np

BS, S, DM, H, DK = 4, 512, 512, 8, 64
NCORES = 8
WCOLS = 512 + 512 + 256 + 512 + 64 + 128  # Wk,Wq,Wv,WoB,bvb,id = 1984
FCOLS = 8 + 128  # biases + idf


def build_program(nc, tile, mybir):
    f32 = mybir.dt.float32
    bf16 = mybir.dt.bfloat16
    i32 = mybir.dt.int32
    AF = mybir.ActivationFunctionType
    OP = mybir.AluOpType

    wb_d = nc.dram_tensor("wblob", [128, WCOLS], bf16, kind="ExternalInput")
    fb_d = nc.dram_tensor("fblob", [128, FCOLS], f32, kind="ExternalInput")
    kT_d = nc.dram_tensor("kT", [BS, 128, 4, S], bf16, kind="ExternalInput")
    qT_d = nc.dram_tensor("qT", [BS, 128, 4, S], bf16, kind="ExternalInput")
    vT_d = nc.dram_tensor("vT", [BS, 128, 4, S], bf16, kind="ExternalInput")
    outT_d = nc.dram_tensor("outT", [BS, 128, 4, S], bf16, kind="ExternalOutput")

    with tile.TileContext(nc) as tc:
        with (
            tc.tile_pool(name="persist", bufs=1) as pp,
            tc.tile_pool(name="inp", bufs=3) as inp,
            tc.tile_pool(name="ex", bufs=2) as exp_,
            tc.tile_pool(name="wk", bufs=2) as wkp,
            tc.tile_pool(name="st", bufs=1) as stp,
            tc.tile_pool(name="ob", bufs=2) as obp,
            tc.tile_pool(name="psum", bufs=1, space="PSUM") as psp,
        ):
            # ---- persistent SBUF ----
            wb = pp.tile([128, WCOLS], bf16, tag="wb")
            fb = pp.tile([128, FCOLS], f32, tag="fb")
            Wk_s = wb[:, 0:512].rearrange("p (m c) -> p m c", m=4)
            Wq_s = wb[:, 512:1024].rearrange("p (m c) -> p m c", m=4)
            Wv_s = wb[:, 1024:1280].rearrange("p (m c) -> p m c", m=4)
            WoB_s = wb[:, 1280:1792].rearrange("p (m c) -> p m c", m=4)
            bvb_s = wb[:, 1792:1856]
            id_s = wb[:, 1856:1984]
            bk2 = fb[:, 0:1]
            bq2 = fb[:, 1:2]
            bo4 = fb[:, 2:6]
            alx = fb[:, 6:7]
            b4x = fb[:, 7:8]
            idf_s = fb[:, 8:136]

            khT2 = pp.tile([128, BS, S], bf16, tag="khT2")
            qhT2 = pp.tile([128, BS, S], bf16, tag="qhT2")
            vh_all = pp.tile([128, 4, BS, DK], bf16, tag="vh")  # [j,jc,c,d]
            heads = pp.tile([128, BS, S], bf16, tag="heads")  # 0:64 ln, 64: qh
            Z_all = pp.tile([128, BS, 16], f32, tag="Z")   # (b, ic*4+c)
            Q_all = pp.tile([128, BS, 16], f32, tag="Q")
            w1_all = pp.tile([128, BS, 16], f32, tag="w1")
            w0_all = pp.tile([128, 16], bf16, tag="w0")
            wup = pp.tile([1, 8], f32, tag="wup")

            def psum_pe():
                return psp.tile([128, 2, S], f32, tag="pe", bufs=2, name="ppe")

            def psum_fs(shape):
                return psp.tile(shape, f32, tag="fs", bufs=2, name="pfs")

            def psum_po(shape):
                return psp.tile(shape, f32, tag="po", bufs=2, name="ppo")

            # ---- prelude: ACT table preload + DMAs in use-order ----
            nc.vector.memset(wup[:], 1.0)
            nc.scalar.activation(wup[:], wup[:], AF.Exp)
            wmm = pp.tile([128, 128], bf16, tag="wmm")
            nc.gpsimd.memset(wmm[:], 0.0)
            nc.sync.dma_start(wb[:], wb_d[:])
            nc.sync.dma_start(fb[:], fb_d[:])
            ktiles = [inp.tile([128, 4, S], bf16, tag="kt", bufs=4,
                               name=f"kt{b}") for b in range(BS)]
            qtiles = [inp.tile([128, 4, S], bf16, tag="qt", bufs=4,
                               name=f"qt{b}") for b in range(BS)]
            vtiles = [inp.tile([128, 4, S], bf16, tag="vt", bufs=4,
                               name=f"vt{b}") for b in range(BS)]
            # trigger order = sync-engine issue order: wblob first, then k
            # (P2 needs all of kh), q0, then v and the remaining q's.
            nc.sync.dma_start(ktiles[0][:], kT_d[0])
            nc.sync.dma_start(ktiles[1][:], kT_d[1])
            nc.sync.dma_start(qtiles[0][:], qT_d[0])
            nc.sync.dma_start(ktiles[2][:], kT_d[2])
            nc.sync.dma_start(ktiles[3][:], kT_d[3])
            nc.sync.dma_start(qtiles[1][:], qT_d[1])
            nc.sync.dma_start(qtiles[2][:], qT_d[2])
            nc.sync.dma_start(qtiles[3][:], qT_d[3])
            nc.sync.dma_start(vtiles[0][:], vT_d[0])
            nc.sync.dma_start(vtiles[1][:], vT_d[1])
            nc.sync.dma_start(vtiles[2][:], vT_d[2])
            nc.sync.dma_start(vtiles[3][:], vT_d[3])

            # ---- phase emitters ----
            def emit_p1(W_s, bias_c, src, dst, b, alt=False):
                ps = (psp.tile([128, S], f32, tag="pe", bufs=2, name="ppe")
                      if alt else psum_po([128, S]))
                for mc in range(4):
                    nc.tensor.matmul(ps[:], W_s[:, mc, :], src[:, mc, :],
                                     start=(mc == 0), stop=(mc == 3))
                nc.vector.tensor_scalar(dst[:, b, :], ps[:], bias_c, None,
                                        op0=OP.add)

            def emit_hq(b):
                nc.vector.tensor_copy(heads[64:128, b, :], qhT2[64:128, b, :])

            def emit_vh(c):
                pv = psum_po([128, 4, DK])
                vt = vtiles[c]
                for jc in range(4):
                    for mc in range(4):
                        nc.tensor.matmul(
                            pv[:, jc, :], vt[:, mc, jc * 128:(jc + 1) * 128],
                            Wv_s[:, mc, :], start=(mc == 0), stop=(mc == 3))
                nc.vector.tensor_tensor(
                    vh_all[:, :, c, :], pv[:],
                    bvb_s[:].unsqueeze(1).broadcast_to((128, 4, DK)),
                    op=OP.add)

            rtiles = {}

            extiles = {}

            def emit_p2h(b, jc, half):
                jcs = slice(jc * 128, (jc + 1) * 128)
                if half == 0:
                    extiles[(b, jc)] = exp_.tile([128, 4, S], bf16, tag="ex",
                                                 bufs=4, name="exq")
                exq = extiles[(b, jc)]
                ph = psum_pe()
                kb0, kb1 = 2 * half, 2 * half + 1
                nc.tensor.matmul(ph[:, 0, :], khT2[0:64, kb0, jcs],
                                 qhT2[0:64, b, :], start=True, stop=True)
                nc.tensor.matmul(ph[:, 1, :], khT2[64:128, kb1, jcs],
                                 qhT2[64:128, b, :], start=True, stop=True)
                nc.scalar.activation(
                    exq[:, 2 * half:2 * half + 2, :], ph[:], AF.Exp)

            def emit_p2f(b, jc):
                exq = extiles[(b, jc)]
                fp = psum_fs([128, S])
                for kb in range(4):
                    nc.tensor.matmul(fp[:], id_s[:], exq[:, kb, :],
                                     start=(kb == 0), stop=(kb == 3))
                wrec = wkp.tile([128, S], f32, tag="wrec", bufs=3, name="wrec")
                nc.vector.reciprocal_approx_fast(wrec[:], fp[:])
                if jc == 0:
                    rtiles[b] = wkp.tile([128, 4, S], bf16, tag="rt",
                                         bufs=4, name=f"rt{b}")
                nc.gpsimd.tensor_tensor(rtiles[b][:, jc, :], exq[:, b, :],
                                        wrec[:], op=OP.mult)

            def emit_p2(b, jc):
                emit_p2h(b, jc, 0)
                emit_p2h(b, jc, 1)
                emit_p2f(b, jc)

            etiles = {}

            def emit_p3(b, p):
                rt = rtiles[b]
                if p == 0:
                    etiles[b] = wkp.tile([128, 4, BS, DK], bf16, tag="ea",
                                         bufs=4, name=f"ea{b}")
                ea = etiles[b]
                scp = psum_fs([128, 2, BS * DK])
                for u in range(2):
                    ic = 2 * p + u
                    ics = slice(ic * 128, (ic + 1) * 128)
                    for jc in range(4):
                        nc.tensor.matmul(
                            scp[:, u, :], rt[:, jc, ics],
                            vh_all[:, jc].rearrange("p c d -> p (c d)"),
                            start=(jc == 0), stop=(jc == 3))
                nc.scalar.activation(
                    ea[:, 2 * p:2 * p + 2].rearrange("p a c d -> p a (c d)"),
                    scp[:], AF.Exp)

            def emit_zq(b, eng):
                # e2 = e*e (gpsimd), then grouped reduces -> Z, Q (DVE-only)
                ea = etiles[b]
                e2 = wkp.tile([128, 16, DK], bf16, tag="e2", name="e2")
                eav = ea[:].rearrange("p a c d -> p (a c) d")
                eng.tensor_tensor(e2[:], eav, eav, op=OP.mult)
                nc.vector.tensor_reduce(Z_all[:, b, :], eav,
                                        axis=mybir.AxisListType.X, op=OP.add)
                nc.vector.tensor_reduce(Q_all[:, b, :], e2[:],
                                        axis=mybir.AxisListType.X, op=OP.add)

            SQ63 = float(np.sqrt(63.0))

            def emit_stats(b0, nb):
                # batches [b0, b0+nb): ln scale-invariance kills 1/Z:
                # w1 = sqrt(63)*rsqrt(Q - Z^2/64), w0 = -Z*w1/64
                bsl = slice(b0, b0 + nb)
                cnt = [128, 16 * nb]
                Zv = Z_all[:, bsl, :].rearrange("p b g -> p (b g)")
                Qv = Q_all[:, bsl, :].rearrange("p b g -> p (b g)")
                t_ = stp.tile(cnt, f32, tag=f"t{b0}", name="t_")
                nc.vector.tensor_tensor(t_[:], Zv, Zv, op=OP.mult)
                s_ = stp.tile(cnt, f32, tag=f"s{b0}", name="s_")
                nc.vector.scalar_tensor_tensor(s_[:], t_[:], -1.0 / DK, Qv,
                                               op0=OP.mult, op1=OP.add)
                # rsqrt seed + 1 NR iter (w1 fused into the final mult)
                r_ = stp.tile(cnt, f32, tag=f"r{b0}", name="r_")
                nc.vector.tensor_scalar(r_[:].bitcast(i32), s_[:].bitcast(i32),
                                        1, None, op0=OP.logical_shift_right)
                nc.vector.tensor_scalar(r_[:].bitcast(i32), r_[:].bitcast(i32),
                                        -1, 0x5F3759DF, op0=OP.mult, op1=OP.add)
                nt = stp.tile(cnt, f32, tag=f"n{b0}", name="nt")
                nc.vector.tensor_tensor(nt[:], s_[:], r_[:], op=OP.mult)
                nc.vector.tensor_tensor(nt[:], nt[:], r_[:], op=OP.mult)
                nc.vector.tensor_scalar(nt[:], nt[:], -0.5, 1.5,
                                        op0=OP.mult, op1=OP.add)
                w1v = w1_all[:, bsl, :].rearrange("p b g -> p (b g)")
                nc.vector.scalar_tensor_tensor(w1v, r_[:], SQ63, nt[:],
                                               op0=OP.mult, op1=OP.mult)
                zg = stp.tile(cnt, f32, tag=f"zg{b0}", name="zg")
                nc.vector.scalar_tensor_tensor(zg[:], Zv, -1.0 / DK, w1v,
                                               op0=OP.mult, op1=OP.mult)
                w0r = stp.tile([128, 4 * nb], f32, tag=f"w{b0}", name="w0r")
                nc.vector.tensor_reduce(
                    w0r[:], zg[:].rearrange("p (g c) -> p g c", c=4),
                    axis=mybir.AxisListType.X, op=OP.add)
                nc.vector.tensor_copy(w0_all[:, 4 * b0:4 * (b0 + nb)], w0r[:])

            bsctiles = {}

            def emit_bsc(b, eng):
                ea = etiles[b]
                bsc = obp.tile([128, 16, DK], bf16, tag="bsc", bufs=4,
                               name=f"bsc{b}")
                bsctiles[b] = bsc
                w1b = (w1_all[:, b, :].unsqueeze(-1)
                       .broadcast_to((128, 16, DK)))
                eng.tensor_tensor(
                    bsc[:], ea[:].rearrange("p a c d -> p (a c) d"), w1b,
                    op=OP.mult)

            def emit_p5(b, heads_act=True):
                bsc = bsctiles[b]
                bp = psum_fs([128, 4, DK])
                for ic in range(4):
                    for c in range(4):
                        nc.tensor.matmul(bp[:, ic, :], id_s[:],
                                         bsc[:, ic * 4 + c, :],
                                         start=(c == 0), stop=False)
                    # += w0 broadcast along d via a 0-stride rhs
                    nc.tensor.matmul(
                        bp[:, ic, :], id_s[:],
                        w0_all[:, b * 4 + ic:b * 4 + ic + 1]
                        .broadcast_to((128, DK)),
                        start=False, stop=True)
                balls = obp.tile([128, 4, DK], f32, tag="balls", name="balls")
                nc.vector.tensor_copy(balls[:], bp[:])
                pt = psum_po([64, S])
                for ic in range(4):
                    nc.tensor.matmul(pt[0:64, ic * 128:(ic + 1) * 128],
                                     balls[:, ic, :], idf_s,
                                     is_transpose=True, start=True, stop=True)
                if heads_act:
                    nc.scalar.activation(heads[0:64, b, :], pt[0:64, :],
                                         AF.Identity, bias=b4x[0:64, :],
                                         scale=alx[0:64, :])
                else:
                    nc.vector.tensor_scalar(heads[0:64, b, :], pt[0:64, :],
                                            alx[0:64, :], b4x[0:64, :],
                                            op0=OP.mult, op1=OP.add)

            def emit_p6(b, act_all=False):
                osb = obp.tile([128, 4, S], bf16, tag="osb", name="osb")
                for half in range(2):
                    pp6 = psp.tile([128, 2, S], f32, tag="pe", bufs=2,
                                   name="pp6")
                    for u in range(2):
                        nch = 2 * half + u
                        nc.tensor.matmul(pp6[:, u, :], WoB_s[:, nch, :],
                                         heads[:, b, :], start=True, stop=True)
                    sl2 = slice(2 * half, 2 * half + 2)
                    if act_all or half == 0:
                        nc.scalar.activation(osb[:, sl2, :], pp6[:], AF.Copy)
                    else:
                        nc.vector.tensor_copy(osb[:, sl2, :], pp6[:])
                nc.sync.dma_start(outT_d[b], osb[:])

            # ---- emission schedule (engine queues are FIFO) ----
            # PE warmup (HAM) on junk data while DMA streams
            for _ in range(44):
                pw_ = psp.tile([128, 128], f32, tag="po", bufs=2, name="pwm")
                nc.tensor.matmul(pw_[:], wmm[:], wmm[:], start=True, stop=True)
            emit_p1(Wk_s, bk2, ktiles[0], khT2, 0)
            emit_p1(Wk_s, bk2, ktiles[1], khT2, 1, alt=True)
            emit_p1(Wq_s, bq2, qtiles[0], qhT2, 0)
            emit_hq(0)
            emit_p2h(0, 0, 0)
            emit_p2h(0, 1, 0)
            emit_p1(Wk_s, bk2, ktiles[2], khT2, 2, alt=True)
            emit_p2h(0, 2, 0)
            emit_p1(Wk_s, bk2, ktiles[3], khT2, 3)
            emit_p2h(0, 3, 0)
            emit_p2h(0, 0, 1)
            emit_p2f(0, 0)
            emit_p1(Wq_s, bq2, qtiles[1], qhT2, 1)
            emit_hq(1)
            emit_p2h(0, 1, 1)
            emit_p2f(0, 1)
            emit_p2h(0, 2, 1)
            emit_p2f(0, 2)
            emit_p1(Wq_s, bq2, qtiles[2], qhT2, 2)
            emit_hq(2)
            emit_p2h(0, 3, 1)
            emit_p2f(0, 3)
            emit_p2(1, 0)
            emit_p1(Wq_s, bq2, qtiles[3], qhT2, 3)
            emit_hq(3)
            emit_p2(1, 1)
            emit_p2(1, 2)
            emit_p2(1, 3)
            emit_vh(0)
            emit_vh(1)
            emit_p2(2, 0)
            emit_vh(2)
            emit_p2(2, 1)
            emit_vh(3)
            emit_p3(0, 0)
            emit_p2(2, 2)
            emit_p3(0, 1)
            emit_p2(2, 3)
            emit_p3(1, 0)
            emit_p2(3, 0)
            emit_p3(1, 1)
            emit_p2(3, 1)
            emit_p3(2, 0)
            emit_p2(3, 2)
            emit_p3(2, 1)
            emit_p2(3, 3)
            emit_zq(0, nc.vector)
            emit_stats(0, 1)
            emit_bsc(0, nc.gpsimd)
            emit_p3(3, 0)
            emit_p3(3, 1)
            emit_p5(0)
            emit_zq(1, nc.vector)
            emit_stats(1, 1)
            emit_p6(0, act_all=True)
            emit_bsc(1, nc.gpsimd)
            emit_p5(1)
            emit_zq(2, nc.vector)
            emit_stats(2, 1)
            emit_p6(1, act_all=True)
            emit_bsc(2, nc.gpsimd)
            emit_p5(2)
            emit_zq(3, nc.vector)
            emit_stats(3, 1)
            emit_p6(2, act_all=True)
            emit_bsc(3, nc.gpsimd)
            emit_p5(3)
            emit_p6(3, act_all=True)

    return nc


def _build():
    import concourse.bass as bass  # noqa
    import concourse.tile as tile
    from concourse import bacc, mybir

    nc = bacc.Bacc("TRN2", target_bir_lowering=False, debug=False,
                   num_devices=NCORES)
    build_program(nc, tile, mybir)
    nc.compile()
    return nc


_cached_nc = None


def make_in_maps(q, k, v, Wq, bq, Wk, bk, Wv, bv, Wo, bo, alpha, beta):
    import ml_dtypes
    bft = ml_dtypes.bfloat16

    def prelay(x):
        xT = np.swapaxes(np.asarray(x, np.float32), 1, 2)  # [B, DM, S]
        return np.ascontiguousarray(
            xT.reshape(BS, 4, 128, S).transpose(0, 2, 1, 3)).astype(bft)

    def wlay(W):  # [DM, DK] -> [128, 4, DK]
        return np.ascontiguousarray(
            np.asarray(W, np.float32).reshape(4, 128, DK).transpose(1, 0, 2))

    qT, kT, vT = prelay(q), prelay(k), prelay(v)
    Wq, Wk, Wv, Wo = (np.asarray(x, np.float32) for x in (Wq, Wk, Wv, Wo))
    bq, bk, bv, bo = (np.asarray(x, np.float32) for x in (bq, bk, bv, bo))
    alpha, beta = np.asarray(alpha, np.float32), np.asarray(beta, np.float32)
    scale = np.float32(1.0 / np.sqrt(np.float32(DK)))
    idbf = np.eye(128, dtype=np.float32)
    in_maps = []
    for h in range(NCORES):
        sl = slice(h * DK, (h + 1) * DK)
        WkD = wlay(Wk[:, sl])
        WkD = np.concatenate([WkD, WkD], axis=2).reshape(128, 512)
        WqD = wlay(Wq[:, sl])
        WqD = np.concatenate([WqD, WqD], axis=2).reshape(128, 512)
        WvS = wlay(Wv[:, sl] * scale).reshape(128, 256)
        WoStack = np.concatenate([Wo[sl, :], 4.0 * Wo[sl, :]], axis=0)
        WoB = WoStack.reshape(128, 4, 128).reshape(128, 512)
        bvb = np.tile((bv[sl] * scale)[None, :], (128, 1))
        wblob = np.ascontiguousarray(np.concatenate(
            [WkD, WqD, WvS, WoB, bvb, idbf], axis=1)).astype(bft)
        bo_h = bo if h == 0 else np.zeros_like(bo)
        alx = np.zeros(128, np.float32)
        alx[0:DK] = alpha
        b4x = np.zeros(128, np.float32)
        b4x[0:DK] = 4.0 * beta
        fblob = np.ascontiguousarray(np.concatenate(
            [np.tile(bk[sl], 2)[:, None], np.tile(bq[sl], 2)[:, None],
             bo_h.reshape(4, 128).T, alx[:, None], b4x[:, None], idbf],
            axis=1)).astype(np.float32)
        in_maps.append({
            "wblob": wblob, "fblob": fblob,
            "kT": kT, "qT": qT, "vT": vT,
        })
    return in_maps


def assemble(results, bo):
    out = np.zeros((BS, S, DM), np.float32)
    for r in results:
        out += np.asarray(r["outT"], np.float32).transpose(0, 3, 2, 1).reshape(
            BS, S, DM)
    return out + np.asarray(bo, np.float32)


def kernel(**inputs) -> np.ndarray:
    global _cached_nc
    from concourse.bass_utils import run_bass_kernel_spmd

    if _cached_nc is None:
        _cached_nc = _build()
    in_maps = make_in_maps(**inputs)
    res = run_bass_kernel_spmd(_cached_nc, in_maps, list(range(NCORES)))
    return assemble(res.results, inputs["bo"])
